# revision 86
# baseline (speedup 1.0000x reference)
"""Multi-head attention (RoPE) forward for Trainium2, 8 NeuronCores.

Problem: B=2, S=2048, D=1024, H=16 heads, Dh=64, fp32 in/out.

Sharding (host side): data-parallel over the 2 batches x 4-way tensor
parallel over heads -> each of the 8 cores handles (batch b, 4 heads) with
its column slice of wq/wk/wv and row slice of wo. Each core returns a
partial output out[b].T contribution; the host sums the 4 partials per
batch (the wo row-reduction).

Device kernel (per core), all in "transposed" layout (features on SBUF
partitions, sequence on the free dim) so no on-device transposes are
needed (the host feeds x[b].T):

  qT = (wq_c)^T x^T, kT likewise (PSUM fp32, bf16 operands)
  RoPE via DVE, all in SBUF bf16 (2x mode). The host pre-permutes wq/wk
      columns so rotation pair elements land at partitions j and j+32
      (contiguous blocks; the permutation cancels in q.k) and supplies
      32-row-replicated cos tables plus a SIGN-ALTERNATING sin table
      (+s,-s,+s,-s per 32-row block). Per 512-col block this takes 7 DVE
      ops: qs copy, mc = qs*cos, 4 partition-shifted msx strips (the +-
      signs baked into the table make every combine an ADD), and ONE
      full-128-row combine qT = mc + msx. (A both-SBUF TensorTensor must
      share base partition on this walrus; non-{0,64} bases max 32 rows.)
  v  = x wv_c in natural [S, 256] layout (x^T used as lhsT)
  per (head, 512-query block): for each pair of 128-key blocks:
      scoresT = kT_tile^T qT_block (K=64 contraction, one PSUM bank each)
      probsT  = exp(scoresT / 8)  (ScalarE, 1024-wide straight from PSUM)
      attn^T += [v_tile | 1]^T probsT   (ones column yields the softmax
                                         denominator as attn^T row 64)
  normalize: recip = 1/denominator (DVE); broadcast across 64 partitions
      via a rank-1 ones matmul (PE); PSUM->SBUF copy (ACT for heads 0/1,
      DVE for 2/3 -- balance found by TimelineSim sweep); multiply (DVE)
  outT = wo_c^T attn_out^T (accumulated over the 2 K-blocks); PSUM ->
      bf16 staging -> DMA out (host accumulates partials in fp32).
      Staging tiles are PAIRED ([128,2,512], one DMA per fo pair) because
      the kernel tail is paced by the serial per-transfer HWDGE
      descriptor-generation slots, not by the copies; for the last query
      block the two halves of each pair are staged on different engines
      (DVE/ACT) so a pair completes in one copy-time.

  The RoPE tables are DMA'd as [32, S] and replicated on-device by the
  otherwise-idle ScalarE (scale=-1 copies make the -s blocks), keeping
  the serial input-DMA stream short: wk, x0, wq, tables, wv, x1-3, wo,
  so TensorE's first projections and the v-projection fill the
  DMA-starved start window. (GpSimd extended-ISA ops - partition
  broadcast/reduce - do not compile on this walrus; plain Pool
  TensorTensor compiles but returns garbage on HW, so Pool is unusable
  for compute and everything balances across PE/ACT/DVE.)

The walrus build here accepts only ONE sync wait per instruction; Tile
emits more. _split_multi_waits legalizes the final BIR by hoisting extra
waits onto same-engine NoOps (identical semantics: waits execute on the
engine sequencer in program order).
"""
import sys

for _p in ("/opt/trn_rl_repo",):
    if _p not in sys.path:
        sys.path.insert(0, _p)

import numpy as np
import ml_dtypes

import concourse.bass as bass
import concourse.mybir as mybir
import concourse.tile as tile
import concourse.tile_sem_assignment as _tsa

# 3 engine sems + 6 DMA queues (5-6 measured ~100ns better than 4 in the
# TimelineSim sweep; 7-8 slightly worse).
_tsa.NUM_HWDGE_SEMS = 6

from concourse.bass_utils import run_bass_kernel_spmd

_wsplit_ctr = [0]


def _split_multi_waits(nc, keep="last"):
    """Legalize the BIR for this walrus build (max ONE sync wait per
    instruction): hoist all but one wait of any instruction onto
    same-engine NoOps placed directly before it. Waits execute on the
    engine's sequencer in program order, so this is semantics-preserving.
    keep: which wait stays on the real instruction ("last" or "first") --
    the NoOps' waits block the SEQ while the instruction's own wait parks
    in the non-blocking wait queue, so the choice shifts head-of-line
    blocking."""
    for f in nc.m.functions:
        for bb in f.blocks:
            insts = bb.instructions
            new_list = []
            changed = False
            for inst in insts:
                si = inst.sync_info
                ow = list(si.on_wait) if (si is not None and si.on_wait) else []
                if len(ow) > 1:
                    changed = True
                    if keep == "first":
                        ow = [ow[0]] + ow[1:][::-1]
                        ow = ow[1:] + ow[:1]
                    for w in ow[:-1]:
                        _wsplit_ctr[0] += 1
                        new_list.append(mybir.InstNoOp(
                            name=f"I-wsplit-{_wsplit_ctr[0]}",
                            engine=inst.engine,
                            ins=[], outs=[],
                            sync_info=mybir.SyncInfo(on_wait=[w], on_update=[]),
                        ))
                    inst.sync_info = mybir.SyncInfo(
                        on_wait=[ow[-1]],
                        on_update=list(si.on_update) if si.on_update else [],
                    )
                new_list.append(inst)
            if changed:
                bb.instructions = new_list
    return nc


F32 = mybir.dt.float32
BF16 = mybir.dt.bfloat16
I16 = mybir.dt.int16

B, S, D, H, DH = 2, 2048, 1024, 16, 64
N_CORES = 8
HPC = H // (N_CORES // B)       # 4 heads per core
FPC = HPC * DH                  # 256 features per core
SQ = 512                        # query-block size (free dim of scores matmul)
SK = 128                        # key-block size (partition dim of scoresT)
NSQ = S // SQ                   # 4
NSK = S // SK                   # 16
KO = D // 128                   # 8 contraction blocks for the projections
EXP_SCALE = 1.0 / 8.0           # 1/sqrt(DH)

# DVE fast-exp (Schraudolph, bf16 bit trick): probs = bitcast_bf16(
# int16(score * 128/(ln2*8) + (127*128 + delta))). HW float->int16
# conversion is round-to-nearest (verified); delta = -4.5 centers the
# piecewise-linear 2^frac interpolation error (+-3.5% max, ~2% rms,
# systematic part cancels in the softmax normalization). Only a bounded
# fraction of tiles use this (error adds ~2% * sqrt(fraction) to output).
EXPA = 128.0 / (float(np.log(2.0)) * 8.0)
EXPB = 127.0 * 128.0 - 4.5

# schedule knobs (swept offline with TimelineSim)
CFG = dict(
    dve_exp_p1=0,    # sk2 tiles per pass-1 attn block exp'd on DVE (of 8)
    dve_exp_p2=0,    # ... per pass-2 attn block
    dve_exp_last=0,  # ... per attn block of the LAST sq heads 2/3 (ACT-paced
                     # end era with idle DVE; bounded accuracy cost)
    dve_exp_last01=0,  # ... last sq heads 0/1
    bs_act_p1=False,  # transpose-back/broadcast copy on ACT (else DVE)
    bs_act_p2=False,
    stage_act=0,     # out-proj staging copies routed to ACT (of 8 per sq)
    stage_act_last=4,  # ... additionally for the LAST sq only
    warm_first=False,  # emit PE warm-up before the load DMAs
    dma_variant=2,   # 0: csa/csb right after wq; 1: interleaved with x
                     # 2: wv right after csb; 3: wv between csa and csb
    rope_split0=True,  # split first k/q RoPE combines per head
    emit_variant=1,  # 0: two head-passes; 1: per-sq all-4-heads interleave
    qk_ahead_prio=-400000,   # priority offset for next-sq projections
    v_prio=-300000,          # priority offset for v projections (st 0-7)
    v_prio2=-1000000,        # priority offset for late v projections (8-15)
    out_prio=-2000000,       # priority offset for out-proj fill
    norm_prio=0,             # priority offset for the normalize chain
    psA_bufs=2,      # projection PSUM pool depth
    n_warm=20,       # PE warm-up dummy matmuls
    v_copy_act=0,    # v-proj PSUM->SBUF copies routed to ACT (of 16)
    tables_dve=False,  # replicate RoPE tables on DVE (4x) instead of ACT
    out_split_last=0,  # last-sq out-proj fo groups whose ko0 pre-accumulates
    prb_bufs=16,     # probs SBUF pool depth
    tmp_bufs=3,      # scratch SBUF pool depth
    ost_bufs=12,     # out-stage SBUF pool depth
    warm_tiny=False,  # 1-row warm-up operands (faster t=0 bootstrap)
    warm_nomemset=False,  # warm-up matmuls on uninitialized SBUF
    head_order=(0, 1, 2, 3),  # per-sq attention block order
    merge_at_bc=False,  # broadcast shares the at PSUM tile rows 64..127
    col_split_last=0,  # column-split normalize+out-proj of the last blocks
    table_rows=32,   # host-provided cos table rows (32, 64 or 128)
    table_rows_b=32,  # host-provided sin table rows (>= table_rows)
    wsplit_keep="last",  # which wait stays on the instruction (see _split)
    attn_pipe=False,  # software-pipelined attn emission order
    pv_nat=True,     # natural-layout PV + per-partition normalize + PE
                     # transpose back (output free size 65 vs 512)
    tp_psA=False,    # transpose PSUM tiles from the proj pool (less churn
                     # on the PV-accumulator pool)
    sk_group=2,      # key tiles per score-PSUM tile / exp instruction
    psS_bufs=2,      # score PSUM pool depth
    psAt_bufs=2,     # PV-accumulator PSUM pool depth
    dma_pairs=True,  # one output DMA per fo pair (halves HWDGE slots)
    tab_late_prio=0,  # deprioritize table replication rows 64-127
)


def _build():
    nc = bass.Bass()
    xT = nc.declare_dram_parameter("xT", [D, S], BF16, isOutput=False)
    wqp = nc.declare_dram_parameter("wq", [D, FPC], BF16, isOutput=False)
    wkp = nc.declare_dram_parameter("wk", [D, FPC], BF16, isOutput=False)
    wvp = nc.declare_dram_parameter("wv", [D, FPC], BF16, isOutput=False)
    wop = nc.declare_dram_parameter("wo", [FPC, D], BF16, isOutput=False)
    TR = CFG["table_rows"]
    TRB = max(TR, CFG["table_rows_b"])
    csap = nc.declare_dram_parameter("csa", [TR, S], BF16, isOutput=False)
    csbp = nc.declare_dram_parameter("csb", [TRB, S], BF16, isOutput=False)
    idp = nc.declare_dram_parameter("ident", [128, 128], BF16, isOutput=False)
    outp = nc.declare_dram_parameter("outT", [D, S], BF16, isOutput=True)

    with tile.TileContext(nc) as tc:
        with tc.tile_pool(name="persist", bufs=1) as pers, \
             tc.tile_pool(name="tmp", bufs=CFG["tmp_bufs"]) as tmp, \
             tc.tile_pool(name="probs", bufs=CFG["prb_bufs"]) as prb, \
             tc.tile_pool(name="ostage", bufs=CFG["ost_bufs"]) as ost, \
             tc.tile_pool(name="psA", bufs=CFG["psA_bufs"], space="PSUM") as psA, \
             tc.tile_pool(name="psS", bufs=CFG["psS_bufs"], space="PSUM") as psS, \
             tc.tile_pool(name="psAt", bufs=max(1, CFG["psAt_bufs"]), space="PSUM") as psAt:

            # ---------------- loads (all into dedicated tiles) -------------
            # order matters: the shared DMA device serializes transfers, so
            # the first qk-projection's inputs (wk + x chunk0, in ko-halves
            # so matmuls can start on the first half) go first; the RoPE
            # tables are only needed ~2 DMAs later.
            warm_in = pers.tile([128, 256], BF16, tag="warm")
            wps_pool = psA if CFG["psAt_bufs"] == 0 else psAt
            wps = wps_pool.tile([128, 256], F32,
                                tag="proj" if CFG["psAt_bufs"] == 0 else "attn",
                                name="warm_ps")

            def warmup():
                # PE warm-up: the HAM clock gate releases only after ~3.4us
                # of sustained PE activity; burn dummy matmuls on a zero tile
                # while the input DMAs are in flight so the real projections
                # run at 2.4 GHz from the start. Lowest priority: these fill
                # TensorE idle slots and keep the HAM activity window hot.
                if CFG["warm_nomemset"]:
                    # read the tile uninitialized: the product is never
                    # consumed (psum cleared by later start=True groups), and
                    # skipping the DVE memset lets PE activity - and the
                    # warm-clock ramp - start ~1.2us earlier
                    lhs, rhs = warm_in[:, 0:128], warm_in[:]
                elif CFG["warm_tiny"]:
                    with tc.high_priority():
                        nc.vector.memset(warm_in[0:1, :], 0.0)
                    lhs, rhs = warm_in[0:1, 0:128], warm_in[0:1, :]
                else:
                    nc.vector.memset(warm_in[:], 0.0)
                    lhs, rhs = warm_in[:, 0:128], warm_in[:]
                with tc.high_priority(offset=-3000000):
                    for _ in range(CFG["n_warm"]):
                        nc.tensor.matmul(wps[0:lhs.shape[1], :] if CFG["warm_tiny"] else wps[:],
                                         lhs, rhs, start=True, stop=True)
                nc.vector.memset(warm_in[0:1, 0:1], 0.0)

            if CFG["warm_first"]:
                warmup()

            xT_sb = pers.tile([128, KO, S], BF16, tag="xT")
            xTr = xT.rearrange("(ko p) s -> p ko s", p=128)
            wk_sb = pers.tile([128, KO, FPC], BF16, tag="wk")
            nc.sync.dma_start(wk_sb[:], wkp.rearrange("(ko p) m -> p ko m", p=128))

            def load_x(xc):
                for kh in range(2):
                    ks = bass.ts(kh, KO // 2)
                    nc.sync.dma_start(xT_sb[:, ks, bass.ts(xc, SQ)],
                                      xTr[:, ks, bass.ts(xc, SQ)])

            csa_sb = pers.tile([128, S], BF16, tag="csa")
            csb_sb = pers.tile([128, S], BF16, tag="csb")
            wv_sb = pers.tile([128, KO, FPC], BF16, tag="wv")
            wq_sb = pers.tile([128, KO, FPC], BF16, tag="wq")

            def load_wq():
                nc.sync.dma_start(wq_sb[:],
                                  wqp.rearrange("(ko p) m -> p ko m", p=128))

            if CFG["dma_variant"] == 4:
                # wq lands between the two x0 halves: the q projection's
                # first ko-half can start while k's second half still loads
                nc.sync.dma_start(xT_sb[:, 0:KO // 2, bass.ts(0, SQ)],
                                  xTr[:, 0:KO // 2, bass.ts(0, SQ)])
                load_wq()
                nc.sync.dma_start(xT_sb[:, KO // 2:KO, bass.ts(0, SQ)],
                                  xTr[:, KO // 2:KO, bass.ts(0, SQ)])
            else:
                load_x(0)
                load_wq()

            def load_tables():
                # the tables are 64-row periodic on-device ([cos;cos] and
                # [+s;-s]): DMA [TR, S] host-stacked rows and replicate the
                # rest with the otherwise-idle ScalarE (a scale=-1 copy
                # makes -s blocks when starting from [32, S]). ACT copy cost
                # depends on free size only, so fewer, taller copies win.
                nc.sync.dma_start(csa_sb[0:TR, :], csap[:])
                nc.sync.dma_start(csb_sb[0:TRB, :], csbp[:])
                CP = mybir.ActivationFunctionType.Copy
                # replication on the otherwise-idle ScalarE; csa/csb copies
                # INTERLEAVED (csa-r1, csb-r1, ...) so the first RoPE's
                # cos and +-sin rows both become available earliest.
                if TR == 32 and TRB == 32:
                    for r in range(1, 4):
                        sgn = -1.0 if r % 2 else 1.0
                        nc.scalar.activation(csa_sb[bass.ts(r, 32), :],
                                             csa_sb[0:32, :], CP)
                        nc.scalar.activation(csb_sb[bass.ts(r, 32), :],
                                             csb_sb[0:32, :], CP, scale=sgn)
                else:
                    if TR == 32:
                        for r in range(1, 4):
                            nc.scalar.activation(csa_sb[bass.ts(r, 32), :],
                                                 csa_sb[0:32, :], CP)
                    elif TR == 64:
                        nc.scalar.activation(csa_sb[64:128, :],
                                             csa_sb[0:64, :], CP)
                    if TRB == 32:
                        for r in range(1, 4):
                            sgn = -1.0 if r % 2 else 1.0
                            nc.scalar.activation(csb_sb[bass.ts(r, 32), :],
                                                 csb_sb[0:32, :], CP,
                                                 scale=sgn)
                    elif TRB == 64:
                        nc.scalar.activation(csb_sb[64:128, :],
                                             csb_sb[0:64, :], CP)

            def load_wv():
                nc.sync.dma_start(wv_sb[:],
                                  wvp.rearrange("(ko p) m -> p ko m", p=128))

            v = CFG["dma_variant"]
            if v == 0:
                load_tables()
                for xc in range(1, NSQ):
                    load_x(xc)
                load_wv()
            elif v == 1:
                load_x(1)
                load_tables()
                load_x(2)
                load_x(3)
                load_wv()
            elif v in (2, 4):
                load_tables()
                load_wv()
                for xc in range(1, NSQ):
                    load_x(xc)
            else:
                load_tables()
                load_wv()
                for xc in range(1, NSQ):
                    load_x(xc)
            wo_sb = pers.tile([128, FPC // 128, D], BF16, tag="wo")
            nc.sync.dma_start(wo_sb[:], wop.rearrange("(ko p) m -> p ko m", p=128))
            id_sb = pers.tile([128, 128], BF16, tag="ident")
            if CFG["pv_nat"]:
                nc.sync.dma_start(id_sb[:], idp[:])

            if not CFG["warm_first"]:
                warmup()

            # ones column for the denominator broadcast matmul
            ones_sb = pers.tile([1, DH], BF16, tag="ones")
            nc.vector.memset(ones_sb[:], 1.0)

            # persistent activations
            qT = [pers.tile([128, S], BF16, tag=f"qT{ft}", name=f"qT{ft}")
                  for ft in range(2)]
            kT = [pers.tile([128, S], BF16, tag=f"kT{ft}", name=f"kT{ft}")
                  for ft in range(2)]
            # [v | 1] as PV stationary tiles: per (sk, head) a [128, DH+1]
            v_sb = pers.tile([128, NSK, HPC, DH + 1], BF16, tag="v")
            nc.vector.memset(v_sb[:, :, :, DH:], 1.0)
            # attention output (bf16, feeds the out-projection)
            aT = [pers.tile([128, S], BF16, tag=f"aT{ft}", name=f"aT{ft}")
                  for ft in range(2)]

            # ---------------- v projection (natural layout) ---------------
            def v_proj(st):
                ps = psA.tile([128, FPC], F32, tag="proj", name="vproj_ps")
                for ko in range(KO):
                    nc.tensor.matmul(
                        ps[:],
                        xT_sb[:, ko, bass.ts(st, 128)],
                        wv_sb[:, ko, :],
                        start=(ko == 0), stop=(ko == KO - 1),
                    )
                if st < CFG["v_copy_act"]:
                    # ScalarE is idle during the start window; keeping these
                    # copies off DVE (busy with RoPE) frees psA slots sooner
                    nc.scalar.copy(
                        v_sb[:, st, :, 0:DH],
                        ps.rearrange("p (h d) -> p h d", h=HPC))
                else:
                    nc.vector.tensor_copy(
                        v_sb[:, st, :, 0:DH],
                        ps.rearrange("p (h d) -> p h d", h=HPC))

            # ---------------- q/k projection + RoPE ------------------------
            # psum rows per head offset: [t0 (32) ; t1 (32)]. One PSUM->SBUF
            # bf16 copy, then 6 SBUF ops at the DVE 2x rate:
            #   mc       = qs * cos_rep                       (128 rows)
            #   msx[ 0:32 ] = qs[32:64 ] * csb[32:64 ]  (= -t1*s: csb row
            #   msx[32:64 ] = qs[ 0:32 ] * csb[ 0:32 ]   blocks alternate
            #   msx[64:96 ] = qs[96:128] * csb[96:128]   +s,-s,+s,-s so all
            #   msx[96:128] = qs[64:96 ] * csb[64:96 ]   combines are ADDs)
            #   dst      = mc + msx                           (128 rows)
            # (partition patterns at base 32/96 are limited to 32 partitions
            # on this walrus, hence the 32-aligned strips; both SBUF inputs
            # of a TensorTensor must share their base partition, the output
            # may differ)
            def qk_proj(w_sb, dst, ft, sq, split_heads=False):
                sl = bass.ts(sq, SQ)
                ps = psA.tile([128, SQ], F32, tag="proj", name="qkproj_ps")
                for ko in range(KO):
                    nc.tensor.matmul(
                        ps[:],
                        w_sb[:, ko, bass.ts(ft, 128)],
                        xT_sb[:, ko, bass.ts(sq, SQ)],
                        start=(ko == 0), stop=(ko == KO - 1),
                    )
                qs = tmp.tile([128, SQ], BF16, tag="ropeQS")
                nc.vector.tensor_copy(qs[:], ps[:])
                mc = tmp.tile([128, SQ], BF16, tag="ropeMC")
                msx = tmp.tile([128, SQ], BF16, tag="ropeMSX")
                if split_heads:
                    # per-head chains so the first head's scores can issue
                    # before the second head's RoPE finishes (start latency)
                    nc.vector.tensor_mul(mc[0:64, :], qs[0:64, :],
                                         csa_sb[0:64, sl])
                    nc.vector.tensor_mul(msx[0:32, :], qs[32:64, :],
                                         csb_sb[32:64, sl])
                    nc.vector.tensor_mul(msx[32:64, :], qs[0:32, :],
                                         csb_sb[0:32, sl])
                    nc.vector.tensor_add(dst[0:64, sl], mc[0:64, :],
                                         msx[0:64, :])
                    nc.vector.tensor_mul(mc[64:128, :], qs[64:128, :],
                                         csa_sb[64:128, sl])
                    nc.vector.tensor_mul(msx[64:96, :], qs[96:128, :],
                                         csb_sb[96:128, sl])
                    nc.vector.tensor_mul(msx[96:128, :], qs[64:96, :],
                                         csb_sb[64:96, sl])
                    nc.vector.tensor_add(dst[64:128, sl], mc[64:128, :],
                                         msx[64:128, :])
                    return
                nc.vector.tensor_mul(mc[:], qs[:], csa_sb[:, sl])
                nc.vector.tensor_mul(msx[0:32, :], qs[32:64, :],
                                     csb_sb[32:64, sl])
                nc.vector.tensor_mul(msx[32:64, :], qs[0:32, :],
                                     csb_sb[0:32, sl])
                nc.vector.tensor_mul(msx[64:96, :], qs[96:128, :],
                                     csb_sb[96:128, sl])
                nc.vector.tensor_mul(msx[96:128, :], qs[64:96, :],
                                     csb_sb[64:96, sl])
                nc.vector.tensor_add(dst[:, sl], mc[:], msx[:])

            # ---------------- attention block ------------------------------
            def attn_block(sq, h, n_dve_exp=0, bs_act=True, col_split=False):
                sl = bass.ts(sq, SQ)
                ft, off = h // 2, (h % 2) * 64
                if CFG["pv_nat"]:
                    at = None   # natural-PV path allocates its own psum
                elif CFG["merge_at_bc"]:
                    # one 128-partition tile per block: PV accumulates into
                    # rows 0..64 and the ones-broadcast matmul reuses rows
                    # 64..127 (the reciprocal reads the denominator row
                    # before the broadcast overwrites it). Keeps bc from
                    # occupying a second psAt slot, so block n+1's PV can
                    # start while block n's normalize still runs.
                    at = psAt.tile([128, SQ], F32, tag="attn")
                else:
                    at = psAt.tile([DH + 1, SQ], F32, tag="attn")
                # spread the DVE-exp'd tiles across the block
                dve_tiles = {NSK // 2 - 1 - 2 * j for j in range(n_dve_exp)}
                def emit_sc_exp_g(sks, dve):
                    # one score tile + ONE exp instruction for a GROUP of
                    # key tiles (bigger groups amortize the per-exp access
                    # overhead and slot-recycle pitch on ScalarE)
                    g = len(sks)
                    GW = CFG["sk_group"]
                    sc = psS.tile([128, GW, SQ], F32, tag="sc")
                    pb = prb.tile([128, GW, SQ], BF16, tag="pb")
                    for i, sk in enumerate(sks):
                        nc.tensor.matmul(
                            sc[:, i, :],
                            kT[ft][off:off + 64, bass.ts(sk, SK)],
                            qT[ft][off:off + 64, sl],
                            start=True, stop=True,
                        )
                    if dve:
                        with nc.allow_low_precision(reason="fast exp"):
                            nc.vector.tensor_scalar(
                                pb[:, 0:g, :].bitcast(I16), sc[:, 0:g, :],
                                EXPA, EXPB,
                                mybir.AluOpType.mult, mybir.AluOpType.add)
                    else:
                        nc.scalar.activation(
                            pb[:, 0:g, :], sc[:, 0:g, :],
                            mybir.ActivationFunctionType.Exp, scale=EXP_SCALE)
                    return pb

                def emit_sc_exp(sk2):
                    return emit_sc_exp_g([2 * sk2, 2 * sk2 + 1],
                                         sk2 in dve_tiles)

                def emit_pv(sk2, pb):
                    for i in range(2):
                        sk = 2 * sk2 + i
                        nc.tensor.matmul(
                            at[0:DH + 1, :], v_sb[:, sk, h, :], pb[:, i, :],
                            start=(sk == 0), stop=(sk == NSK - 1),
                        )

                def emit_pv_nat(sk2, pb, atn):
                    # natural-layout PV: probs is the STATIONARY operand so
                    # the output is [128 queries, DH+1] -- free size 65
                    # instead of 512, 4x cheaper on TensorE per element.
                    # PSUM start=True zeroes the WHOLE 2KB bank
                    # (ZERO_REGION_SIZE), so only the very first matmul may
                    # carry it: the other query-subtiles' first writes
                    # accumulate onto the already-zeroed bank.
                    for i in range(2):
                        sk = 2 * sk2 + i
                        for qs4 in range(4):
                            nc.tensor.matmul(
                                atn[:, qs4, :],
                                pb[:, i, bass.ts(qs4, 128)],
                                v_sb[:, sk, h, :],
                                start=(sk == 0 and qs4 == 0),
                                stop=(sk == NSK - 1),
                                skip_group_check=True,
                            )

                if CFG["pv_nat"]:
                    atn_pool = psA if CFG["psAt_bufs"] == 0 else psAt
                    atn = atn_pool.tile(
                        [128, 4, DH + 1], F32,
                        tag="proj" if CFG["psAt_bufs"] == 0 else "attn",
                        name="at_nat")
                    GW = CFG["sk_group"]
                    groups = [list(range(s, min(s + GW, NSK)))
                              for s in range(0, NSK, GW)]
                    for gi, sks in enumerate(groups):
                        pbs = emit_sc_exp_g(sks, False)
                        for i, sk in enumerate(sks):
                            for qs4 in range(4):
                                nc.tensor.matmul(
                                    atn[:, qs4, :],
                                    pbs[:, i, bass.ts(qs4, 128)],
                                    v_sb[:, sk, h, :],
                                    start=(sk == 0 and qs4 == 0),
                                    stop=(sk == NSK - 1),
                                    skip_group_check=True,
                                )
                    for qs4 in range(4):
                        # per-partition normalize (queries on partitions):
                        # no broadcast needed at all
                        rcn = tmp.tile([128, 1], F32, tag="recip", name="rcn")
                        with nc.allow_low_precision(
                                reason="softmax denominator"):
                            nc.vector.reciprocal(
                                rcn[:], atn[:, qs4, DH:DH + 1])
                        ann = tmp.tile([128, DH], BF16, tag="anat",
                                       name="ann")
                        nc.vector.tensor_scalar(
                            ann[:], atn[:, qs4, 0:DH], rcn[:], None,
                            mybir.AluOpType.mult)
                        # transpose back to [features, queries] for the
                        # out-projection (PE transpose mode, bf16)
                        use_psA = CFG["tp_psA"] or CFG["psAt_bufs"] == 0
                        tp = (psA if use_psA else psAt).tile(
                            [DH, 128], BF16,
                            tag="proj" if use_psA else "attn",
                            name="tp_ps")
                        nc.tensor.transpose(tp[:], ann[:], id_sb[:])
                        csl = bass.ts(4 * sq + qs4, 128)
                        if bs_act:
                            nc.scalar.copy(aT[ft][off:off + 64, csl], tp[:])
                        else:
                            nc.vector.tensor_copy(aT[ft][off:off + 64, csl],
                                                  tp[:])
                    return
                if CFG["attn_pipe"]:
                    # software-pipelined emission: next tile's scores sit
                    # ahead of this tile's PV in the tie-break order
                    pbs = emit_sc_exp(0)
                    for sk2 in range(1, NSK // 2):
                        pb_next = emit_sc_exp(sk2)
                        emit_pv(sk2 - 1, pbs)
                        pbs = pb_next
                    emit_pv(NSK // 2 - 1, pbs)
                else:
                    for sk2 in range(NSK // 2):
                        pbs = emit_sc_exp(sk2)
                        emit_pv(sk2, pbs)
                ctx = tc.high_priority(offset=CFG["norm_prio"]) \
                    if CFG["norm_prio"] else None
                if ctx is not None:
                    ctx.__enter__()
                # col_split: run the normalize per column half so the first
                # half of the (column-split) out-projection can start while
                # the second half still normalizes -- shortens the epilogue
                # of the final attention block.
                SH = SQ // 2
                halves = ((0, SH), (SH, SH)) if col_split else ((0, SQ),)
                for c0, cw in halves:
                    cs = slice(c0, c0 + cw)
                    sls = bass.ts(2 * sq + c0 // SH, SH) if col_split else sl
                    rc = tmp.tile([1, cw], BF16, tag="recip", name="rc")
                    with nc.allow_low_precision(reason="softmax denominator"):
                        nc.vector.reciprocal(rc[:], at[DH:DH + 1, cs])
                    if CFG["merge_at_bc"]:
                        bc = at[DH:2 * DH, cs]
                        nc.tensor.matmul(bc, ones_sb[:], rc[:],
                                         start=True, stop=True,
                                         skip_group_check=True)
                    else:
                        bct = psAt.tile([DH, cw], F32, tag="attn",
                                        name="bcast_ps")
                        bc = bct[:]
                        nc.tensor.matmul(bc, ones_sb[:], rc[:],
                                         start=True, stop=True)
                    bs = tmp.tile([DH, cw], F32, tag="bcsb", name="bs")
                    if bs_act:
                        nc.scalar.copy(bs[:], bc)
                    else:
                        nc.vector.tensor_copy(bs[:], bc)
                    nc.vector.tensor_mul(aT[ft][off:off + 64, sls],
                                         at[0:DH, cs], bs[:])
                if ctx is not None:
                    ctx.__exit__(None, None, None)

            # ---------------- out-projection for one query block -----------
            outpR = outp.rearrange("(fo p) s -> p fo s", p=128)

            def out_proj_finish(sq, fo, po, stg=None):
                sl = bass.ts(sq, SQ)
                on_act = fo < CFG["stage_act"]
                if sq == NSQ - 1 and fo % 2 == 1 and \
                        fo < 2 * CFG["stage_act_last"]:
                    on_act = True
                if stg is None:
                    stg1 = ost.tile([128, SQ], BF16, tag="oT", name="stg1")
                    dst = stg1[:]
                else:
                    stg1 = None
                    dst = stg
                if on_act:
                    nc.scalar.copy(dst, po[:])
                else:
                    nc.vector.tensor_copy(dst, po[:])
                if stg1 is not None:
                    nc.sync.dma_start(outp[bass.ts(fo, 128), sl], dst)

            def out_proj_ko(sq, fo, po, ko, start, stop):
                nc.tensor.matmul(
                    po[:],
                    wo_sb[:, ko, bass.ts(fo, 128)],
                    aT[ko][:, bass.ts(sq, SQ)],
                    start=start, stop=stop,
                )

            def out_proj(sq, skip_fo=()):
                last = sq == NSQ - 1
                pair = CFG["dma_pairs"]
                csplit = last and CFG["col_split_last"]
                sl = bass.ts(sq, SQ)
                SH = SQ // 2
                stg2 = None
                for fo in range(8):
                    if fo in skip_fo:
                        continue
                    # on the last block the scores stream is done, so its
                    # PSUM pool is free: borrow it for 2 extra po slots
                    if last and fo % 2 == 1:
                        po = psS.tile([128, SQ], F32, tag="sc", name="oproj_ps2")
                    else:
                        po = psA.tile([128, SQ], F32, tag="proj", name="oproj_ps")
                    if csplit:
                        # column-split: the first half contracts aT columns
                        # that finish normalizing earlier
                        for ch in range(2):
                            ccs = slice(ch * SH, (ch + 1) * SH)
                            for ko in range(2):
                                nc.tensor.matmul(
                                    po[:, ccs],
                                    wo_sb[:, ko, bass.ts(fo, 128)],
                                    aT[ko][:, bass.ts(2 * sq + ch, SH)],
                                    start=(ko == 0), stop=(ko == 1),
                                )
                    else:
                        out_proj_ko(sq, fo, po, 0, True, False)
                        out_proj_ko(sq, fo, po, 1, False, True)
                    if not pair:
                        out_proj_finish(sq, fo, po)
                        continue
                    # paired staging: two fo blocks share one [128,2,SQ]
                    # tile and ONE output DMA (halves the serial HWDGE
                    # descriptor-generation slots that pace the tail)
                    if fo % 2 == 0:
                        stg2 = ost.tile([128, 2, SQ], BF16, tag="oT")
                        dsts = stg2[:, 0, :]
                    else:
                        dsts = stg2[:, 1, :]
                    if csplit:
                        # stage per column half (alternating engines) so
                        # the first half's copy runs during the second
                        # half's matmuls
                        for ch in range(2):
                            ccs = slice(ch * SH, (ch + 1) * SH)
                            if (fo + ch) % 2 == 0:
                                nc.vector.tensor_copy(dsts[:, ccs],
                                                      po[:, ccs])
                            else:
                                nc.scalar.copy(dsts[:, ccs], po[:, ccs])
                    else:
                        out_proj_finish(sq, fo, po, stg=dsts)
                    if fo % 2 == 1:
                        nc.sync.dma_start(outpR[:, fo - 1:fo + 1, sl],
                                          stg2[:])

            # ---------------- emission order (overlap) ---------------------
            if CFG["emit_variant"] == 0:
                # two head-passes: heads 0,1 for all sq, then 2,3 + out-proj
                qk_proj(wk_sb, kT[0], 0, 0, split_heads=CFG["rope_split0"])
                qk_proj(wq_sb, qT[0], 0, 0, split_heads=CFG["rope_split0"])
                for sq in range(1, NSQ):
                    qk_proj(wk_sb, kT[0], 0, sq)
                with tc.high_priority(offset=-400000):
                    for sq in range(1, NSQ):
                        qk_proj(wq_sb, qT[0], 0, sq)
                with tc.high_priority(offset=CFG["v_prio"]):
                    for st in range(8):
                        v_proj(st)
                with tc.high_priority(offset=-1000000):
                    for st in range(8, NSK):
                        v_proj(st)
                for sq in range(NSQ):
                    attn_block(sq, 0, CFG["dve_exp_p1"], CFG["bs_act_p1"])
                    attn_block(sq, 1, CFG["dve_exp_p1"], CFG["bs_act_p1"])
                with tc.high_priority(offset=-500000):
                    for sq in range(NSQ):
                        qk_proj(wk_sb, kT[1], 1, sq)
                    for sq in range(NSQ):
                        qk_proj(wq_sb, qT[1], 1, sq)
                for sq in range(NSQ):
                    attn_block(sq, 2, CFG["dve_exp_p2"], CFG["bs_act_p2"])
                    attn_block(sq, 3, CFG["dve_exp_p2"], CFG["bs_act_p2"])
                    with tc.high_priority(offset=CFG["out_prio"]):
                        out_proj(sq)
            else:
                # per-sq: all 4 heads of each query block back-to-back, with
                # the next block's projections + v + out-proj as PE fill --
                # balances the ACT exp stream across the whole kernel span.
                qk_proj(wk_sb, kT[0], 0, 0, split_heads=CFG["rope_split0"])
                qk_proj(wq_sb, qT[0], 0, 0, split_heads=CFG["rope_split0"])
                with tc.high_priority(offset=CFG["qk_ahead_prio"]):
                    qk_proj(wk_sb, kT[1], 1, 0)
                    qk_proj(wq_sb, qT[1], 1, 0)
                with tc.high_priority(offset=CFG["v_prio"]):
                    for st in range(8):
                        v_proj(st)
                with tc.high_priority(offset=-1000000):
                    for st in range(8, NSK):
                        v_proj(st)
                ho = CFG["head_order"]
                for sq in range(NSQ):
                    last = sq == NSQ - 1
                    nsplit = CFG["out_split_last"] if last else 0
                    de1 = CFG["dve_exp_last01"] if last else CFG["dve_exp_p1"]
                    attn_block(sq, ho[0], de1, CFG["bs_act_p1"])
                    attn_block(sq, ho[1], de1, CFG["bs_act_p1"])
                    if sq + 1 < NSQ:
                        with tc.high_priority(offset=CFG["qk_ahead_prio"]):
                            qk_proj(wk_sb, kT[0], 0, sq + 1)
                            qk_proj(wq_sb, qT[0], 0, sq + 1)
                    # last sq: psA is otherwise idle now, so pre-accumulate
                    # the ko0 half (reads aT[0] = heads 0,1, already final)
                    # of the first fo groups; only ko1+stage+DMA remain
                    # after the last head's normalize.
                    pre = []
                    for fo in range(nsplit):
                        po = psA.tile([128, SQ], F32, tag="proj",
                                      name="oproj_ps")
                        out_proj_ko(sq, fo, po, 0, True, False)
                        pre.append((fo, po))
                    de2 = CFG["dve_exp_last"] if last else CFG["dve_exp_p2"]
                    ncs = CFG["col_split_last"] if last else 0
                    attn_block(sq, ho[2], de2, CFG["bs_act_p2"],
                               col_split=(ncs >= 2))
                    attn_block(sq, ho[3], de2, CFG["bs_act_p2"],
                               col_split=(ncs >= 1))
                    if sq + 1 < NSQ:
                        with tc.high_priority(offset=CFG["qk_ahead_prio"]):
                            qk_proj(wk_sb, kT[1], 1, sq + 1)
                            qk_proj(wq_sb, qT[1], 1, sq + 1)
                    with tc.high_priority(offset=CFG["out_prio"]):
                        for fo, po in pre:
                            out_proj_ko(sq, fo, po, 1, False, True)
                            out_proj_finish(sq, fo, po)
                        out_proj(sq, skip_fo=tuple(f for f, _ in pre))

    _split_multi_waits(nc, keep=CFG["wsplit_keep"])
    return nc


_NC_CACHE = None


def _get_nc():
    global _NC_CACHE
    if _NC_CACHE is None:
        _NC_CACHE = _build()
    return _NC_CACHE


# rotation-pair permutation: within each head, [0,2,...,62, 1,3,...,63]
_PAIR_PERM = np.concatenate([np.arange(0, DH, 2), np.arange(1, DH, 2)])


def kernel(x, freqs_cos, freqs_sin, wq, wk, wv, wo):
    x = np.asarray(x, dtype=np.float32)
    cosT = np.asarray(freqs_cos, np.float32).T    # [32, S]
    sinT = np.asarray(freqs_sin, np.float32).T
    # host-stacked base tables (the kernel replicates the rest on-device;
    # the on-device pattern is [cos]x4 and [+s,-s,+s,-s] per 32-row block)
    TR = CFG["table_rows"]
    TRB = max(TR, CFG["table_rows_b"])
    csa = np.ascontiguousarray(
        np.concatenate([cosT] * (TR // 32), 0)).astype(ml_dtypes.bfloat16)
    sgn = [sinT if r % 2 == 0 else -sinT for r in range(TRB // 32)]
    csb = np.ascontiguousarray(np.concatenate(sgn, 0)).astype(
        ml_dtypes.bfloat16)
    wq = np.asarray(wq, np.float32)
    wk = np.asarray(wk, np.float32)
    wv = np.asarray(wv, np.float32)
    wo = np.asarray(wo, np.float32)

    bf = ml_dtypes.bfloat16
    in_maps = []
    for c in range(N_CORES):
        b, hg = divmod(c, N_CORES // B)
        heads = [hg * HPC + i for i in range(HPC)]
        qk_cols = np.concatenate([h * DH + _PAIR_PERM for h in heads])
        v_cols = np.concatenate([h * DH + np.arange(DH) for h in heads])
        in_maps.append({
            "ident": np.eye(128, dtype=np.float32).astype(bf),
            "xT": np.ascontiguousarray(x[b].T).astype(bf),
            "wq": np.ascontiguousarray(wq[:, qk_cols]).astype(bf),
            "wk": np.ascontiguousarray(wk[:, qk_cols]).astype(bf),
            "wv": np.ascontiguousarray(wv[:, v_cols]).astype(bf),
            "wo": np.ascontiguousarray(wo[v_cols, :]).astype(bf),
            "csa": csa,
            "csb": csb,
        })

    nc = _get_nc()
    res = run_bass_kernel_spmd(nc, in_maps, core_ids=list(range(N_CORES)))

    out = np.zeros((B, S, D), dtype=np.float32)
    for c in range(N_CORES):
        b = c // (N_CORES // B)
        out[b] += res.results[c]["outT"].astype(np.float32).T
    return out



# revision 87
# speedup vs baseline: 1.0059x; 1.0059x over previous
"""Multi-head attention (RoPE) forward for Trainium2, 8 NeuronCores.

Problem: B=2, S=2048, D=1024, H=16 heads, Dh=64, fp32 in/out.

Sharding (host side): data-parallel over the 2 batches x 4-way tensor
parallel over heads -> each of the 8 cores handles (batch b, 4 heads) with
its column slice of wq/wk/wv and row slice of wo. Each core returns a
partial output out[b].T contribution; the host sums the 4 partials per
batch (the wo row-reduction).

Device kernel (per core), all in "transposed" layout (features on SBUF
partitions, sequence on the free dim) so no on-device transposes are
needed (the host feeds x[b].T):

  qT = (wq_c)^T x^T, kT likewise (PSUM fp32, bf16 operands)
  RoPE via DVE, all in SBUF bf16 (2x mode). The host pre-permutes wq/wk
      columns so rotation pair elements land at partitions j and j+32
      (contiguous blocks; the permutation cancels in q.k) and supplies
      32-row-replicated cos tables plus a SIGN-ALTERNATING sin table
      (+s,-s,+s,-s per 32-row block). Per 512-col block this takes 7 DVE
      ops: qs copy, mc = qs*cos, 4 partition-shifted msx strips (the +-
      signs baked into the table make every combine an ADD), and ONE
      full-128-row combine qT = mc + msx. (A both-SBUF TensorTensor must
      share base partition on this walrus; non-{0,64} bases max 32 rows.)
  v  = x wv_c in natural [S, 256] layout (x^T used as lhsT)
  per (head, 512-query block): for each pair of 128-key blocks:
      scoresT = kT_tile^T qT_block (K=64 contraction, one PSUM bank each)
      probsT  = exp(scoresT / 8)  (ScalarE, 1024-wide straight from PSUM)
      attn^T += [v_tile | 1]^T probsT   (ones column yields the softmax
                                         denominator as attn^T row 64)
  normalize: recip = 1/denominator (DVE); broadcast across 64 partitions
      via a rank-1 ones matmul (PE); PSUM->SBUF copy (ACT for heads 0/1,
      DVE for 2/3 -- balance found by TimelineSim sweep); multiply (DVE)
  outT = wo_c^T attn_out^T (accumulated over the 2 K-blocks); PSUM ->
      bf16 staging -> DMA out (host accumulates partials in fp32).
      Staging tiles are PAIRED ([128,2,512], one DMA per fo pair) because
      the kernel tail is paced by the serial per-transfer HWDGE
      descriptor-generation slots, not by the copies; for the last query
      block the two halves of each pair are staged on different engines
      (DVE/ACT) so a pair completes in one copy-time.

  The RoPE tables are DMA'd as [32, S] and replicated on-device by the
  otherwise-idle ScalarE (scale=-1 copies make the -s blocks), keeping
  the serial input-DMA stream short: wk, x0, wq, tables, wv, x1-3, wo,
  so TensorE's first projections and the v-projection fill the
  DMA-starved start window. (GpSimd extended-ISA ops - partition
  broadcast/reduce - do not compile on this walrus; plain Pool
  TensorTensor compiles but returns garbage on HW, so Pool is unusable
  for compute and everything balances across PE/ACT/DVE.)

The walrus build here accepts only ONE sync wait per instruction; Tile
emits more. _split_multi_waits legalizes the final BIR by hoisting extra
waits onto same-engine NoOps (identical semantics: waits execute on the
engine sequencer in program order).
"""
import sys

for _p in ("/opt/trn_rl_repo",):
    if _p not in sys.path:
        sys.path.insert(0, _p)

import numpy as np
import ml_dtypes

import concourse.bass as bass
import concourse.mybir as mybir
import concourse.tile as tile
import concourse.tile_sem_assignment as _tsa

# 3 engine sems + 6 DMA queues (5-6 measured ~100ns better than 4 in the
# TimelineSim sweep; 7-8 slightly worse).
_tsa.NUM_HWDGE_SEMS = 6

from concourse.bass_utils import run_bass_kernel_spmd

_wsplit_ctr = [0]


def _split_multi_waits(nc, keep="last"):
    """Legalize the BIR for this walrus build (max ONE sync wait per
    instruction): hoist all but one wait of any instruction onto
    same-engine NoOps placed directly before it. Waits execute on the
    engine's sequencer in program order, so this is semantics-preserving.
    keep: which wait stays on the real instruction ("last" or "first") --
    the NoOps' waits block the SEQ while the instruction's own wait parks
    in the non-blocking wait queue, so the choice shifts head-of-line
    blocking."""
    for f in nc.m.functions:
        for bb in f.blocks:
            insts = bb.instructions
            new_list = []
            changed = False
            for inst in insts:
                si = inst.sync_info
                ow = list(si.on_wait) if (si is not None and si.on_wait) else []
                if len(ow) > 1:
                    changed = True
                    if keep == "first":
                        ow = [ow[0]] + ow[1:][::-1]
                        ow = ow[1:] + ow[:1]
                    for w in ow[:-1]:
                        _wsplit_ctr[0] += 1
                        new_list.append(mybir.InstNoOp(
                            name=f"I-wsplit-{_wsplit_ctr[0]}",
                            engine=inst.engine,
                            ins=[], outs=[],
                            sync_info=mybir.SyncInfo(on_wait=[w], on_update=[]),
                        ))
                    inst.sync_info = mybir.SyncInfo(
                        on_wait=[ow[-1]],
                        on_update=list(si.on_update) if si.on_update else [],
                    )
                new_list.append(inst)
            if changed:
                bb.instructions = new_list
    return nc


F32 = mybir.dt.float32
BF16 = mybir.dt.bfloat16
I16 = mybir.dt.int16

B, S, D, H, DH = 2, 2048, 1024, 16, 64
N_CORES = 8
HPC = H // (N_CORES // B)       # 4 heads per core
FPC = HPC * DH                  # 256 features per core
SQ = 512                        # query-block size (free dim of scores matmul)
SK = 128                        # key-block size (partition dim of scoresT)
NSQ = S // SQ                   # 4
NSK = S // SK                   # 16
KO = D // 128                   # 8 contraction blocks for the projections
EXP_SCALE = 1.0 / 8.0           # 1/sqrt(DH)

# DVE fast-exp (Schraudolph, bf16 bit trick): probs = bitcast_bf16(
# int16(score * 128/(ln2*8) + (127*128 + delta))). HW float->int16
# conversion is round-to-nearest (verified); delta = -4.5 centers the
# piecewise-linear 2^frac interpolation error (+-3.5% max, ~2% rms,
# systematic part cancels in the softmax normalization). Only a bounded
# fraction of tiles use this (error adds ~2% * sqrt(fraction) to output).
EXPA = 128.0 / (float(np.log(2.0)) * 8.0)
EXPB = 127.0 * 128.0 - 4.5

# schedule knobs (swept offline with TimelineSim)
CFG = dict(
    dve_exp_p1=0,    # sk2 tiles per pass-1 attn block exp'd on DVE (of 8)
    dve_exp_p2=0,    # ... per pass-2 attn block
    dve_exp_last=0,  # ... per attn block of the LAST sq heads 2/3 (ACT-paced
                     # end era with idle DVE; bounded accuracy cost)
    dve_exp_last01=0,  # ... last sq heads 0/1
    bs_act_p1=False,  # transpose-back/broadcast copy on ACT (else DVE)
    bs_act_p2=False,
    stage_act=0,     # out-proj staging copies routed to ACT (of 8 per sq)
    stage_act_last=4,  # ... additionally for the LAST sq only
    warm_first=False,  # emit PE warm-up before the load DMAs
    dma_variant=2,   # 0: csa/csb right after wq; 1: interleaved with x
                     # 2: wv right after csb; 3: wv between csa and csb
    rope_split0=True,  # split first k/q RoPE combines per head
    emit_variant=1,  # 0: two head-passes; 1: per-sq all-4-heads interleave
    qk_ahead_prio=-400000,   # priority offset for next-sq projections
    v_prio=-300000,          # priority offset for v projections (st 0-7)
    v_prio2=-1000000,        # priority offset for late v projections (8-15)
    out_prio=-2000000,       # priority offset for out-proj fill
    norm_prio=0,             # priority offset for the normalize chain
    psA_bufs=2,      # projection PSUM pool depth
    n_warm=20,       # PE warm-up dummy matmuls
    v_copy_act=0,    # v-proj PSUM->SBUF copies routed to ACT (of 16)
    tables_dve=False,  # replicate RoPE tables on DVE (4x) instead of ACT
    out_split_last=0,  # last-sq out-proj fo groups whose ko0 pre-accumulates
    prb_bufs=20,     # probs SBUF pool depth
    tmp_bufs=3,      # scratch SBUF pool depth
    ost_bufs=12,     # out-stage SBUF pool depth
    warm_tiny=False,  # 1-row warm-up operands (faster t=0 bootstrap)
    warm_nomemset=False,  # warm-up matmuls on uninitialized SBUF
    head_order=(0, 1, 2, 3),  # per-sq attention block order
    merge_at_bc=False,  # broadcast shares the at PSUM tile rows 64..127
    col_split_last=0,  # column-split normalize+out-proj of the last blocks
    table_rows=32,   # host-provided cos table rows (32, 64 or 128)
    table_rows_b=32,  # host-provided sin table rows (>= table_rows)
    wsplit_keep="last",  # which wait stays on the instruction (see _split)
    attn_pipe=False,  # software-pipelined attn emission order
    pv_nat=True,     # natural-layout PV + per-partition normalize + PE
                     # transpose back (output free size 65 vs 512)
    tp_psA=False,    # transpose PSUM tiles from the proj pool (less churn
                     # on the PV-accumulator pool)
    sk_group=2,      # key tiles per score-PSUM tile / exp instruction
    psS_bufs=2,      # score PSUM pool depth
    psAt_bufs=2,     # PV-accumulator PSUM pool depth
    dma_pairs=True,  # one output DMA per fo pair (halves HWDGE slots)
    tab_late_prio=0,  # deprioritize table replication rows 64-127
)


def _build():
    nc = bass.Bass()
    xT = nc.declare_dram_parameter("xT", [D, S], BF16, isOutput=False)
    wqp = nc.declare_dram_parameter("wq", [D, FPC], BF16, isOutput=False)
    wkp = nc.declare_dram_parameter("wk", [D, FPC], BF16, isOutput=False)
    wvp = nc.declare_dram_parameter("wv", [D, FPC], BF16, isOutput=False)
    wop = nc.declare_dram_parameter("wo", [FPC, D], BF16, isOutput=False)
    TR = CFG["table_rows"]
    TRB = max(TR, CFG["table_rows_b"])
    csap = nc.declare_dram_parameter("csa", [TR, S], BF16, isOutput=False)
    csbp = nc.declare_dram_parameter("csb", [TRB, S], BF16, isOutput=False)
    idp = nc.declare_dram_parameter("ident", [128, 128], BF16, isOutput=False)
    outp = nc.declare_dram_parameter("outT", [D, S], BF16, isOutput=True)

    with tile.TileContext(nc) as tc:
        with tc.tile_pool(name="persist", bufs=1) as pers, \
             tc.tile_pool(name="tmp", bufs=CFG["tmp_bufs"]) as tmp, \
             tc.tile_pool(name="probs", bufs=CFG["prb_bufs"]) as prb, \
             tc.tile_pool(name="ostage", bufs=CFG["ost_bufs"]) as ost, \
             tc.tile_pool(name="psA", bufs=CFG["psA_bufs"], space="PSUM") as psA, \
             tc.tile_pool(name="psS", bufs=CFG["psS_bufs"], space="PSUM") as psS, \
             tc.tile_pool(name="psAt", bufs=max(1, CFG["psAt_bufs"]), space="PSUM") as psAt:

            # ---------------- loads (all into dedicated tiles) -------------
            # order matters: the shared DMA device serializes transfers, so
            # the first qk-projection's inputs (wk + x chunk0, in ko-halves
            # so matmuls can start on the first half) go first; the RoPE
            # tables are only needed ~2 DMAs later.
            warm_in = pers.tile([128, 256], BF16, tag="warm")
            wps_pool = psA if CFG["psAt_bufs"] == 0 else psAt
            wps = wps_pool.tile([128, 256], F32,
                                tag="proj" if CFG["psAt_bufs"] == 0 else "attn",
                                name="warm_ps")

            def warmup():
                # PE warm-up: the HAM clock gate releases only after ~3.4us
                # of sustained PE activity; burn dummy matmuls on a zero tile
                # while the input DMAs are in flight so the real projections
                # run at 2.4 GHz from the start. Lowest priority: these fill
                # TensorE idle slots and keep the HAM activity window hot.
                if CFG["warm_nomemset"]:
                    # read the tile uninitialized: the product is never
                    # consumed (psum cleared by later start=True groups), and
                    # skipping the DVE memset lets PE activity - and the
                    # warm-clock ramp - start ~1.2us earlier
                    lhs, rhs = warm_in[:, 0:128], warm_in[:]
                elif CFG["warm_tiny"]:
                    with tc.high_priority():
                        nc.vector.memset(warm_in[0:1, :], 0.0)
                    lhs, rhs = warm_in[0:1, 0:128], warm_in[0:1, :]
                else:
                    nc.vector.memset(warm_in[:], 0.0)
                    lhs, rhs = warm_in[:, 0:128], warm_in[:]
                with tc.high_priority(offset=-3000000):
                    for _ in range(CFG["n_warm"]):
                        nc.tensor.matmul(wps[0:lhs.shape[1], :] if CFG["warm_tiny"] else wps[:],
                                         lhs, rhs, start=True, stop=True)
                nc.vector.memset(warm_in[0:1, 0:1], 0.0)

            if CFG["warm_first"]:
                warmup()

            xT_sb = pers.tile([128, KO, S], BF16, tag="xT")
            xTr = xT.rearrange("(ko p) s -> p ko s", p=128)
            wk_sb = pers.tile([128, KO, FPC], BF16, tag="wk")
            nc.sync.dma_start(wk_sb[:], wkp.rearrange("(ko p) m -> p ko m", p=128))

            def load_x(xc):
                for kh in range(2):
                    ks = bass.ts(kh, KO // 2)
                    nc.sync.dma_start(xT_sb[:, ks, bass.ts(xc, SQ)],
                                      xTr[:, ks, bass.ts(xc, SQ)])

            csa_sb = pers.tile([128, S], BF16, tag="csa")
            csb_sb = pers.tile([128, S], BF16, tag="csb")
            wv_sb = pers.tile([128, KO, FPC], BF16, tag="wv")
            wq_sb = pers.tile([128, KO, FPC], BF16, tag="wq")

            def load_wq():
                nc.sync.dma_start(wq_sb[:],
                                  wqp.rearrange("(ko p) m -> p ko m", p=128))

            if CFG["dma_variant"] == 4:
                # wq lands between the two x0 halves: the q projection's
                # first ko-half can start while k's second half still loads
                nc.sync.dma_start(xT_sb[:, 0:KO // 2, bass.ts(0, SQ)],
                                  xTr[:, 0:KO // 2, bass.ts(0, SQ)])
                load_wq()
                nc.sync.dma_start(xT_sb[:, KO // 2:KO, bass.ts(0, SQ)],
                                  xTr[:, KO // 2:KO, bass.ts(0, SQ)])
            else:
                load_x(0)
                load_wq()

            def load_tables():
                # the tables are 64-row periodic on-device ([cos;cos] and
                # [+s;-s]): DMA [TR, S] host-stacked rows and replicate the
                # rest with the otherwise-idle ScalarE (a scale=-1 copy
                # makes -s blocks when starting from [32, S]). ACT copy cost
                # depends on free size only, so fewer, taller copies win.
                nc.sync.dma_start(csa_sb[0:TR, :], csap[:])
                nc.sync.dma_start(csb_sb[0:TRB, :], csbp[:])
                CP = mybir.ActivationFunctionType.Copy
                # replication on the otherwise-idle ScalarE; csa/csb copies
                # INTERLEAVED (csa-r1, csb-r1, ...) so the first RoPE's
                # cos and +-sin rows both become available earliest.
                if TR == 32 and TRB == 32:
                    for r in range(1, 4):
                        sgn = -1.0 if r % 2 else 1.0
                        nc.scalar.activation(csa_sb[bass.ts(r, 32), :],
                                             csa_sb[0:32, :], CP)
                        nc.scalar.activation(csb_sb[bass.ts(r, 32), :],
                                             csb_sb[0:32, :], CP, scale=sgn)
                else:
                    if TR == 32:
                        for r in range(1, 4):
                            nc.scalar.activation(csa_sb[bass.ts(r, 32), :],
                                                 csa_sb[0:32, :], CP)
                    elif TR == 64:
                        nc.scalar.activation(csa_sb[64:128, :],
                                             csa_sb[0:64, :], CP)
                    if TRB == 32:
                        for r in range(1, 4):
                            sgn = -1.0 if r % 2 else 1.0
                            nc.scalar.activation(csb_sb[bass.ts(r, 32), :],
                                                 csb_sb[0:32, :], CP,
                                                 scale=sgn)
                    elif TRB == 64:
                        nc.scalar.activation(csb_sb[64:128, :],
                                             csb_sb[0:64, :], CP)

            def load_wv():
                nc.sync.dma_start(wv_sb[:],
                                  wvp.rearrange("(ko p) m -> p ko m", p=128))

            v = CFG["dma_variant"]
            if v == 0:
                load_tables()
                for xc in range(1, NSQ):
                    load_x(xc)
                load_wv()
            elif v == 1:
                load_x(1)
                load_tables()
                load_x(2)
                load_x(3)
                load_wv()
            elif v in (2, 4):
                load_tables()
                load_wv()
                for xc in range(1, NSQ):
                    load_x(xc)
            else:
                load_tables()
                load_wv()
                for xc in range(1, NSQ):
                    load_x(xc)
            wo_sb = pers.tile([128, FPC // 128, D], BF16, tag="wo")
            nc.sync.dma_start(wo_sb[:], wop.rearrange("(ko p) m -> p ko m", p=128))
            id_sb = pers.tile([128, 128], BF16, tag="ident")
            if CFG["pv_nat"]:
                nc.sync.dma_start(id_sb[:], idp[:])

            if not CFG["warm_first"]:
                warmup()

            # ones column for the denominator broadcast matmul
            ones_sb = pers.tile([1, DH], BF16, tag="ones")
            nc.vector.memset(ones_sb[:], 1.0)

            # persistent activations
            qT = [pers.tile([128, S], BF16, tag=f"qT{ft}", name=f"qT{ft}")
                  for ft in range(2)]
            kT = [pers.tile([128, S], BF16, tag=f"kT{ft}", name=f"kT{ft}")
                  for ft in range(2)]
            # [v | 1] as PV stationary tiles: per (sk, head) a [128, DH+1]
            v_sb = pers.tile([128, NSK, HPC, DH + 1], BF16, tag="v")
            nc.vector.memset(v_sb[:, :, :, DH:], 1.0)
            # attention output (bf16, feeds the out-projection)
            aT = [pers.tile([128, S], BF16, tag=f"aT{ft}", name=f"aT{ft}")
                  for ft in range(2)]

            # ---------------- v projection (natural layout) ---------------
            def v_proj(st):
                ps = psA.tile([128, FPC], F32, tag="proj", name="vproj_ps")
                for ko in range(KO):
                    nc.tensor.matmul(
                        ps[:],
                        xT_sb[:, ko, bass.ts(st, 128)],
                        wv_sb[:, ko, :],
                        start=(ko == 0), stop=(ko == KO - 1),
                    )
                if st < CFG["v_copy_act"]:
                    # ScalarE is idle during the start window; keeping these
                    # copies off DVE (busy with RoPE) frees psA slots sooner
                    nc.scalar.copy(
                        v_sb[:, st, :, 0:DH],
                        ps.rearrange("p (h d) -> p h d", h=HPC))
                else:
                    nc.vector.tensor_copy(
                        v_sb[:, st, :, 0:DH],
                        ps.rearrange("p (h d) -> p h d", h=HPC))

            # ---------------- q/k projection + RoPE ------------------------
            # psum rows per head offset: [t0 (32) ; t1 (32)]. One PSUM->SBUF
            # bf16 copy, then 6 SBUF ops at the DVE 2x rate:
            #   mc       = qs * cos_rep                       (128 rows)
            #   msx[ 0:32 ] = qs[32:64 ] * csb[32:64 ]  (= -t1*s: csb row
            #   msx[32:64 ] = qs[ 0:32 ] * csb[ 0:32 ]   blocks alternate
            #   msx[64:96 ] = qs[96:128] * csb[96:128]   +s,-s,+s,-s so all
            #   msx[96:128] = qs[64:96 ] * csb[64:96 ]   combines are ADDs)
            #   dst      = mc + msx                           (128 rows)
            # (partition patterns at base 32/96 are limited to 32 partitions
            # on this walrus, hence the 32-aligned strips; both SBUF inputs
            # of a TensorTensor must share their base partition, the output
            # may differ)
            def qk_proj(w_sb, dst, ft, sq, split_heads=False):
                sl = bass.ts(sq, SQ)
                ps = psA.tile([128, SQ], F32, tag="proj", name="qkproj_ps")
                for ko in range(KO):
                    nc.tensor.matmul(
                        ps[:],
                        w_sb[:, ko, bass.ts(ft, 128)],
                        xT_sb[:, ko, bass.ts(sq, SQ)],
                        start=(ko == 0), stop=(ko == KO - 1),
                    )
                qs = tmp.tile([128, SQ], BF16, tag="ropeQS")
                nc.vector.tensor_copy(qs[:], ps[:])
                mc = tmp.tile([128, SQ], BF16, tag="ropeMC")
                msx = tmp.tile([128, SQ], BF16, tag="ropeMSX")
                if split_heads:
                    # per-head chains so the first head's scores can issue
                    # before the second head's RoPE finishes (start latency)
                    nc.vector.tensor_mul(mc[0:64, :], qs[0:64, :],
                                         csa_sb[0:64, sl])
                    nc.vector.tensor_mul(msx[0:32, :], qs[32:64, :],
                                         csb_sb[32:64, sl])
                    nc.vector.tensor_mul(msx[32:64, :], qs[0:32, :],
                                         csb_sb[0:32, sl])
                    nc.vector.tensor_add(dst[0:64, sl], mc[0:64, :],
                                         msx[0:64, :])
                    nc.vector.tensor_mul(mc[64:128, :], qs[64:128, :],
                                         csa_sb[64:128, sl])
                    nc.vector.tensor_mul(msx[64:96, :], qs[96:128, :],
                                         csb_sb[96:128, sl])
                    nc.vector.tensor_mul(msx[96:128, :], qs[64:96, :],
                                         csb_sb[64:96, sl])
                    nc.vector.tensor_add(dst[64:128, sl], mc[64:128, :],
                                         msx[64:128, :])
                    return
                nc.vector.tensor_mul(mc[:], qs[:], csa_sb[:, sl])
                nc.vector.tensor_mul(msx[0:32, :], qs[32:64, :],
                                     csb_sb[32:64, sl])
                nc.vector.tensor_mul(msx[32:64, :], qs[0:32, :],
                                     csb_sb[0:32, sl])
                nc.vector.tensor_mul(msx[64:96, :], qs[96:128, :],
                                     csb_sb[96:128, sl])
                nc.vector.tensor_mul(msx[96:128, :], qs[64:96, :],
                                     csb_sb[64:96, sl])
                nc.vector.tensor_add(dst[:, sl], mc[:], msx[:])

            # ---------------- attention block ------------------------------
            def attn_block(sq, h, n_dve_exp=0, bs_act=True, col_split=False):
                sl = bass.ts(sq, SQ)
                ft, off = h // 2, (h % 2) * 64
                if CFG["pv_nat"]:
                    at = None   # natural-PV path allocates its own psum
                elif CFG["merge_at_bc"]:
                    # one 128-partition tile per block: PV accumulates into
                    # rows 0..64 and the ones-broadcast matmul reuses rows
                    # 64..127 (the reciprocal reads the denominator row
                    # before the broadcast overwrites it). Keeps bc from
                    # occupying a second psAt slot, so block n+1's PV can
                    # start while block n's normalize still runs.
                    at = psAt.tile([128, SQ], F32, tag="attn")
                else:
                    at = psAt.tile([DH + 1, SQ], F32, tag="attn")
                # spread the DVE-exp'd tiles across the block
                dve_tiles = {NSK // 2 - 1 - 2 * j for j in range(n_dve_exp)}
                def emit_sc_exp_g(sks, dve):
                    # one score tile + ONE exp instruction for a GROUP of
                    # key tiles (bigger groups amortize the per-exp access
                    # overhead and slot-recycle pitch on ScalarE)
                    g = len(sks)
                    GW = CFG["sk_group"]
                    sc = psS.tile([128, GW, SQ], F32, tag="sc")
                    pb = prb.tile([128, GW, SQ], BF16, tag="pb")
                    for i, sk in enumerate(sks):
                        nc.tensor.matmul(
                            sc[:, i, :],
                            kT[ft][off:off + 64, bass.ts(sk, SK)],
                            qT[ft][off:off + 64, sl],
                            start=True, stop=True,
                        )
                    if dve:
                        with nc.allow_low_precision(reason="fast exp"):
                            nc.vector.tensor_scalar(
                                pb[:, 0:g, :].bitcast(I16), sc[:, 0:g, :],
                                EXPA, EXPB,
                                mybir.AluOpType.mult, mybir.AluOpType.add)
                    else:
                        nc.scalar.activation(
                            pb[:, 0:g, :], sc[:, 0:g, :],
                            mybir.ActivationFunctionType.Exp, scale=EXP_SCALE)
                    return pb

                def emit_sc_exp(sk2):
                    return emit_sc_exp_g([2 * sk2, 2 * sk2 + 1],
                                         sk2 in dve_tiles)

                def emit_pv(sk2, pb):
                    for i in range(2):
                        sk = 2 * sk2 + i
                        nc.tensor.matmul(
                            at[0:DH + 1, :], v_sb[:, sk, h, :], pb[:, i, :],
                            start=(sk == 0), stop=(sk == NSK - 1),
                        )

                def emit_pv_nat(sk2, pb, atn):
                    # natural-layout PV: probs is the STATIONARY operand so
                    # the output is [128 queries, DH+1] -- free size 65
                    # instead of 512, 4x cheaper on TensorE per element.
                    # PSUM start=True zeroes the WHOLE 2KB bank
                    # (ZERO_REGION_SIZE), so only the very first matmul may
                    # carry it: the other query-subtiles' first writes
                    # accumulate onto the already-zeroed bank.
                    for i in range(2):
                        sk = 2 * sk2 + i
                        for qs4 in range(4):
                            nc.tensor.matmul(
                                atn[:, qs4, :],
                                pb[:, i, bass.ts(qs4, 128)],
                                v_sb[:, sk, h, :],
                                start=(sk == 0 and qs4 == 0),
                                stop=(sk == NSK - 1),
                                skip_group_check=True,
                            )

                if CFG["pv_nat"]:
                    atn_pool = psA if CFG["psAt_bufs"] == 0 else psAt
                    atn = atn_pool.tile(
                        [128, 4, DH + 1], F32,
                        tag="proj" if CFG["psAt_bufs"] == 0 else "attn",
                        name="at_nat")
                    GW = CFG["sk_group"]
                    groups = [list(range(s, min(s + GW, NSK)))
                              for s in range(0, NSK, GW)]
                    for gi, sks in enumerate(groups):
                        pbs = emit_sc_exp_g(sks, False)
                        for i, sk in enumerate(sks):
                            for qs4 in range(4):
                                nc.tensor.matmul(
                                    atn[:, qs4, :],
                                    pbs[:, i, bass.ts(qs4, 128)],
                                    v_sb[:, sk, h, :],
                                    start=(sk == 0 and qs4 == 0),
                                    stop=(sk == NSK - 1),
                                    skip_group_check=True,
                                )
                    for qs4 in range(4):
                        # per-partition normalize (queries on partitions):
                        # no broadcast needed at all
                        rcn = tmp.tile([128, 1], F32, tag="recip", name="rcn")
                        with nc.allow_low_precision(
                                reason="softmax denominator"):
                            nc.vector.reciprocal(
                                rcn[:], atn[:, qs4, DH:DH + 1])
                        ann = tmp.tile([128, DH], BF16, tag="anat",
                                       name="ann")
                        nc.vector.tensor_scalar(
                            ann[:], atn[:, qs4, 0:DH], rcn[:], None,
                            mybir.AluOpType.mult)
                        # transpose back to [features, queries] for the
                        # out-projection (PE transpose mode, bf16)
                        use_psA = CFG["tp_psA"] or CFG["psAt_bufs"] == 0
                        tp = (psA if use_psA else psAt).tile(
                            [DH, 128], BF16,
                            tag="proj" if use_psA else "attn",
                            name="tp_ps")
                        nc.tensor.transpose(tp[:], ann[:], id_sb[:])
                        csl = bass.ts(4 * sq + qs4, 128)
                        if bs_act:
                            nc.scalar.copy(aT[ft][off:off + 64, csl], tp[:])
                        else:
                            nc.vector.tensor_copy(aT[ft][off:off + 64, csl],
                                                  tp[:])
                    return
                if CFG["attn_pipe"]:
                    # software-pipelined emission: next tile's scores sit
                    # ahead of this tile's PV in the tie-break order
                    pbs = emit_sc_exp(0)
                    for sk2 in range(1, NSK // 2):
                        pb_next = emit_sc_exp(sk2)
                        emit_pv(sk2 - 1, pbs)
                        pbs = pb_next
                    emit_pv(NSK // 2 - 1, pbs)
                else:
                    for sk2 in range(NSK // 2):
                        pbs = emit_sc_exp(sk2)
                        emit_pv(sk2, pbs)
                ctx = tc.high_priority(offset=CFG["norm_prio"]) \
                    if CFG["norm_prio"] else None
                if ctx is not None:
                    ctx.__enter__()
                # col_split: run the normalize per column half so the first
                # half of the (column-split) out-projection can start while
                # the second half still normalizes -- shortens the epilogue
                # of the final attention block.
                SH = SQ // 2
                halves = ((0, SH), (SH, SH)) if col_split else ((0, SQ),)
                for c0, cw in halves:
                    cs = slice(c0, c0 + cw)
                    sls = bass.ts(2 * sq + c0 // SH, SH) if col_split else sl
                    rc = tmp.tile([1, cw], BF16, tag="recip", name="rc")
                    with nc.allow_low_precision(reason="softmax denominator"):
                        nc.vector.reciprocal(rc[:], at[DH:DH + 1, cs])
                    if CFG["merge_at_bc"]:
                        bc = at[DH:2 * DH, cs]
                        nc.tensor.matmul(bc, ones_sb[:], rc[:],
                                         start=True, stop=True,
                                         skip_group_check=True)
                    else:
                        bct = psAt.tile([DH, cw], F32, tag="attn",
                                        name="bcast_ps")
                        bc = bct[:]
                        nc.tensor.matmul(bc, ones_sb[:], rc[:],
                                         start=True, stop=True)
                    bs = tmp.tile([DH, cw], F32, tag="bcsb", name="bs")
                    if bs_act:
                        nc.scalar.copy(bs[:], bc)
                    else:
                        nc.vector.tensor_copy(bs[:], bc)
                    nc.vector.tensor_mul(aT[ft][off:off + 64, sls],
                                         at[0:DH, cs], bs[:])
                if ctx is not None:
                    ctx.__exit__(None, None, None)

            # ---------------- out-projection for one query block -----------
            outpR = outp.rearrange("(fo p) s -> p fo s", p=128)

            def out_proj_finish(sq, fo, po, stg=None):
                sl = bass.ts(sq, SQ)
                on_act = fo < CFG["stage_act"]
                if sq == NSQ - 1 and fo % 2 == 1 and \
                        fo < 2 * CFG["stage_act_last"]:
                    on_act = True
                if stg is None:
                    stg1 = ost.tile([128, SQ], BF16, tag="oT", name="stg1")
                    dst = stg1[:]
                else:
                    stg1 = None
                    dst = stg
                if on_act:
                    nc.scalar.copy(dst, po[:])
                else:
                    nc.vector.tensor_copy(dst, po[:])
                if stg1 is not None:
                    nc.sync.dma_start(outp[bass.ts(fo, 128), sl], dst)

            def out_proj_ko(sq, fo, po, ko, start, stop):
                nc.tensor.matmul(
                    po[:],
                    wo_sb[:, ko, bass.ts(fo, 128)],
                    aT[ko][:, bass.ts(sq, SQ)],
                    start=start, stop=stop,
                )

            def out_proj(sq, skip_fo=()):
                last = sq == NSQ - 1
                pair = CFG["dma_pairs"]
                csplit = last and CFG["col_split_last"]
                sl = bass.ts(sq, SQ)
                SH = SQ // 2
                stg2 = None
                for fo in range(8):
                    if fo in skip_fo:
                        continue
                    # on the last block the scores stream is done, so its
                    # PSUM pool is free: borrow it for 2 extra po slots
                    if last and fo % 2 == 1:
                        po = psS.tile([128, SQ], F32, tag="sc", name="oproj_ps2")
                    else:
                        po = psA.tile([128, SQ], F32, tag="proj", name="oproj_ps")
                    if csplit:
                        # column-split: the first half contracts aT columns
                        # that finish normalizing earlier
                        for ch in range(2):
                            ccs = slice(ch * SH, (ch + 1) * SH)
                            for ko in range(2):
                                nc.tensor.matmul(
                                    po[:, ccs],
                                    wo_sb[:, ko, bass.ts(fo, 128)],
                                    aT[ko][:, bass.ts(2 * sq + ch, SH)],
                                    start=(ko == 0), stop=(ko == 1),
                                )
                    else:
                        out_proj_ko(sq, fo, po, 0, True, False)
                        out_proj_ko(sq, fo, po, 1, False, True)
                    if not pair:
                        out_proj_finish(sq, fo, po)
                        continue
                    # paired staging: two fo blocks share one [128,2,SQ]
                    # tile and ONE output DMA (halves the serial HWDGE
                    # descriptor-generation slots that pace the tail)
                    if fo % 2 == 0:
                        stg2 = ost.tile([128, 2, SQ], BF16, tag="oT")
                        dsts = stg2[:, 0, :]
                    else:
                        dsts = stg2[:, 1, :]
                    if csplit:
                        # stage per column half (alternating engines) so
                        # the first half's copy runs during the second
                        # half's matmuls
                        for ch in range(2):
                            ccs = slice(ch * SH, (ch + 1) * SH)
                            if (fo + ch) % 2 == 0:
                                nc.vector.tensor_copy(dsts[:, ccs],
                                                      po[:, ccs])
                            else:
                                nc.scalar.copy(dsts[:, ccs], po[:, ccs])
                    else:
                        out_proj_finish(sq, fo, po, stg=dsts)
                    if fo % 2 == 1:
                        nc.sync.dma_start(outpR[:, fo - 1:fo + 1, sl],
                                          stg2[:])

            # ---------------- emission order (overlap) ---------------------
            if CFG["emit_variant"] == 0:
                # two head-passes: heads 0,1 for all sq, then 2,3 + out-proj
                qk_proj(wk_sb, kT[0], 0, 0, split_heads=CFG["rope_split0"])
                qk_proj(wq_sb, qT[0], 0, 0, split_heads=CFG["rope_split0"])
                for sq in range(1, NSQ):
                    qk_proj(wk_sb, kT[0], 0, sq)
                with tc.high_priority(offset=-400000):
                    for sq in range(1, NSQ):
                        qk_proj(wq_sb, qT[0], 0, sq)
                with tc.high_priority(offset=CFG["v_prio"]):
                    for st in range(8):
                        v_proj(st)
                with tc.high_priority(offset=-1000000):
                    for st in range(8, NSK):
                        v_proj(st)
                for sq in range(NSQ):
                    attn_block(sq, 0, CFG["dve_exp_p1"], CFG["bs_act_p1"])
                    attn_block(sq, 1, CFG["dve_exp_p1"], CFG["bs_act_p1"])
                with tc.high_priority(offset=-500000):
                    for sq in range(NSQ):
                        qk_proj(wk_sb, kT[1], 1, sq)
                    for sq in range(NSQ):
                        qk_proj(wq_sb, qT[1], 1, sq)
                for sq in range(NSQ):
                    attn_block(sq, 2, CFG["dve_exp_p2"], CFG["bs_act_p2"])
                    attn_block(sq, 3, CFG["dve_exp_p2"], CFG["bs_act_p2"])
                    with tc.high_priority(offset=CFG["out_prio"]):
                        out_proj(sq)
            else:
                # per-sq: all 4 heads of each query block back-to-back, with
                # the next block's projections + v + out-proj as PE fill --
                # balances the ACT exp stream across the whole kernel span.
                qk_proj(wk_sb, kT[0], 0, 0, split_heads=CFG["rope_split0"])
                qk_proj(wq_sb, qT[0], 0, 0, split_heads=CFG["rope_split0"])
                with tc.high_priority(offset=CFG["qk_ahead_prio"]):
                    qk_proj(wk_sb, kT[1], 1, 0)
                    qk_proj(wq_sb, qT[1], 1, 0)
                with tc.high_priority(offset=CFG["v_prio"]):
                    for st in range(8):
                        v_proj(st)
                with tc.high_priority(offset=-1000000):
                    for st in range(8, NSK):
                        v_proj(st)
                ho = CFG["head_order"]
                for sq in range(NSQ):
                    last = sq == NSQ - 1
                    nsplit = CFG["out_split_last"] if last else 0
                    de1 = CFG["dve_exp_last01"] if last else CFG["dve_exp_p1"]
                    attn_block(sq, ho[0], de1, CFG["bs_act_p1"])
                    attn_block(sq, ho[1], de1, CFG["bs_act_p1"])
                    if sq + 1 < NSQ:
                        with tc.high_priority(offset=CFG["qk_ahead_prio"]):
                            qk_proj(wk_sb, kT[0], 0, sq + 1)
                            qk_proj(wq_sb, qT[0], 0, sq + 1)
                    # last sq: psA is otherwise idle now, so pre-accumulate
                    # the ko0 half (reads aT[0] = heads 0,1, already final)
                    # of the first fo groups; only ko1+stage+DMA remain
                    # after the last head's normalize.
                    pre = []
                    for fo in range(nsplit):
                        po = psA.tile([128, SQ], F32, tag="proj",
                                      name="oproj_ps")
                        out_proj_ko(sq, fo, po, 0, True, False)
                        pre.append((fo, po))
                    de2 = CFG["dve_exp_last"] if last else CFG["dve_exp_p2"]
                    ncs = CFG["col_split_last"] if last else 0
                    attn_block(sq, ho[2], de2, CFG["bs_act_p2"],
                               col_split=(ncs >= 2))
                    attn_block(sq, ho[3], de2, CFG["bs_act_p2"],
                               col_split=(ncs >= 1))
                    if sq + 1 < NSQ:
                        with tc.high_priority(offset=CFG["qk_ahead_prio"]):
                            qk_proj(wk_sb, kT[1], 1, sq + 1)
                            qk_proj(wq_sb, qT[1], 1, sq + 1)
                    with tc.high_priority(offset=CFG["out_prio"]):
                        for fo, po in pre:
                            out_proj_ko(sq, fo, po, 1, False, True)
                            out_proj_finish(sq, fo, po)
                        out_proj(sq, skip_fo=tuple(f for f, _ in pre))

    _split_multi_waits(nc, keep=CFG["wsplit_keep"])
    return nc


_NC_CACHE = None


def _get_nc():
    global _NC_CACHE
    if _NC_CACHE is None:
        _NC_CACHE = _build()
    return _NC_CACHE


# rotation-pair permutation: within each head, [0,2,...,62, 1,3,...,63]
_PAIR_PERM = np.concatenate([np.arange(0, DH, 2), np.arange(1, DH, 2)])


def kernel(x, freqs_cos, freqs_sin, wq, wk, wv, wo):
    x = np.asarray(x, dtype=np.float32)
    cosT = np.asarray(freqs_cos, np.float32).T    # [32, S]
    sinT = np.asarray(freqs_sin, np.float32).T
    # host-stacked base tables (the kernel replicates the rest on-device;
    # the on-device pattern is [cos]x4 and [+s,-s,+s,-s] per 32-row block)
    TR = CFG["table_rows"]
    TRB = max(TR, CFG["table_rows_b"])
    csa = np.ascontiguousarray(
        np.concatenate([cosT] * (TR // 32), 0)).astype(ml_dtypes.bfloat16)
    sgn = [sinT if r % 2 == 0 else -sinT for r in range(TRB // 32)]
    csb = np.ascontiguousarray(np.concatenate(sgn, 0)).astype(
        ml_dtypes.bfloat16)
    wq = np.asarray(wq, np.float32)
    wk = np.asarray(wk, np.float32)
    wv = np.asarray(wv, np.float32)
    wo = np.asarray(wo, np.float32)

    bf = ml_dtypes.bfloat16
    in_maps = []
    for c in range(N_CORES):
        b, hg = divmod(c, N_CORES // B)
        heads = [hg * HPC + i for i in range(HPC)]
        qk_cols = np.concatenate([h * DH + _PAIR_PERM for h in heads])
        v_cols = np.concatenate([h * DH + np.arange(DH) for h in heads])
        in_maps.append({
            "ident": np.eye(128, dtype=np.float32).astype(bf),
            "xT": np.ascontiguousarray(x[b].T).astype(bf),
            "wq": np.ascontiguousarray(wq[:, qk_cols]).astype(bf),
            "wk": np.ascontiguousarray(wk[:, qk_cols]).astype(bf),
            "wv": np.ascontiguousarray(wv[:, v_cols]).astype(bf),
            "wo": np.ascontiguousarray(wo[v_cols, :]).astype(bf),
            "csa": csa,
            "csb": csb,
        })

    nc = _get_nc()
    res = run_bass_kernel_spmd(nc, in_maps, core_ids=list(range(N_CORES)))

    out = np.zeros((B, S, D), dtype=np.float32)
    for c in range(N_CORES):
        b = c // (N_CORES // B)
        out[b] += res.results[c]["outT"].astype(np.float32).T
    return out



# revision 88
# speedup vs baseline: 1.0071x; 1.0011x over previous
"""Multi-head attention (RoPE) forward for Trainium2, 8 NeuronCores.

Problem: B=2, S=2048, D=1024, H=16 heads, Dh=64, fp32 in/out.

Sharding (host side): data-parallel over the 2 batches x 4-way tensor
parallel over heads -> each of the 8 cores handles (batch b, 4 heads) with
its column slice of wq/wk/wv and row slice of wo. Each core returns a
partial output out[b].T contribution; the host sums the 4 partials per
batch (the wo row-reduction).

Device kernel (per core), all in "transposed" layout (features on SBUF
partitions, sequence on the free dim) so no on-device transposes are
needed (the host feeds x[b].T):

  qT = (wq_c)^T x^T, kT likewise (PSUM fp32, bf16 operands)
  RoPE via DVE, all in SBUF bf16 (2x mode). The host pre-permutes wq/wk
      columns so rotation pair elements land at partitions j and j+32
      (contiguous blocks; the permutation cancels in q.k) and supplies
      32-row-replicated cos tables plus a SIGN-ALTERNATING sin table
      (+s,-s,+s,-s per 32-row block). Per 512-col block this takes 7 DVE
      ops: qs copy, mc = qs*cos, 4 partition-shifted msx strips (the +-
      signs baked into the table make every combine an ADD), and ONE
      full-128-row combine qT = mc + msx. (A both-SBUF TensorTensor must
      share base partition on this walrus; non-{0,64} bases max 32 rows.)
  v  = x wv_c in natural [S, 256] layout (x^T used as lhsT)
  per (head, 512-query block): for each pair of 128-key blocks:
      scoresT = kT_tile^T qT_block (K=64 contraction, one PSUM bank each)
      probsT  = exp(scoresT / 8)  (ScalarE, 1024-wide straight from PSUM)
      attn^T += [v_tile | 1]^T probsT   (ones column yields the softmax
                                         denominator as attn^T row 64)
  normalize: recip = 1/denominator (DVE); broadcast across 64 partitions
      via a rank-1 ones matmul (PE); PSUM->SBUF copy (ACT for heads 0/1,
      DVE for 2/3 -- balance found by TimelineSim sweep); multiply (DVE)
  outT = wo_c^T attn_out^T (accumulated over the 2 K-blocks); PSUM ->
      bf16 staging -> DMA out (host accumulates partials in fp32).
      Staging tiles are PAIRED ([128,2,512], one DMA per fo pair) because
      the kernel tail is paced by the serial per-transfer HWDGE
      descriptor-generation slots, not by the copies; for the last query
      block the two halves of each pair are staged on different engines
      (DVE/ACT) so a pair completes in one copy-time.

  The RoPE tables are DMA'd as [32, S] and replicated on-device by the
  otherwise-idle ScalarE (scale=-1 copies make the -s blocks), keeping
  the serial input-DMA stream short: wk, x0, wq, tables, wv, x1-3, wo,
  so TensorE's first projections and the v-projection fill the
  DMA-starved start window. (GpSimd extended-ISA ops - partition
  broadcast/reduce - do not compile on this walrus; plain Pool
  TensorTensor compiles but returns garbage on HW, so Pool is unusable
  for compute and everything balances across PE/ACT/DVE.)

The walrus build here accepts only ONE sync wait per instruction; Tile
emits more. _split_multi_waits legalizes the final BIR by hoisting extra
waits onto same-engine NoOps (identical semantics: waits execute on the
engine sequencer in program order).
"""
import sys

for _p in ("/opt/trn_rl_repo",):
    if _p not in sys.path:
        sys.path.insert(0, _p)

import numpy as np
import ml_dtypes

import concourse.bass as bass
import concourse.mybir as mybir
import concourse.tile as tile
import concourse.tile_sem_assignment as _tsa

# 3 engine sems + 4 DMA queues (re-tuned after the natural-PV
# restructure: 4 and 8 tie at best, 6 is +200ns).
_tsa.NUM_HWDGE_SEMS = 4

from concourse.bass_utils import run_bass_kernel_spmd

_wsplit_ctr = [0]


def _split_multi_waits(nc, keep="last"):
    """Legalize the BIR for this walrus build (max ONE sync wait per
    instruction): hoist all but one wait of any instruction onto
    same-engine NoOps placed directly before it. Waits execute on the
    engine's sequencer in program order, so this is semantics-preserving.
    keep: which wait stays on the real instruction ("last" or "first") --
    the NoOps' waits block the SEQ while the instruction's own wait parks
    in the non-blocking wait queue, so the choice shifts head-of-line
    blocking."""
    for f in nc.m.functions:
        for bb in f.blocks:
            insts = bb.instructions
            new_list = []
            changed = False
            for inst in insts:
                si = inst.sync_info
                ow = list(si.on_wait) if (si is not None and si.on_wait) else []
                if len(ow) > 1:
                    changed = True
                    if keep == "first":
                        ow = [ow[0]] + ow[1:][::-1]
                        ow = ow[1:] + ow[:1]
                    for w in ow[:-1]:
                        _wsplit_ctr[0] += 1
                        new_list.append(mybir.InstNoOp(
                            name=f"I-wsplit-{_wsplit_ctr[0]}",
                            engine=inst.engine,
                            ins=[], outs=[],
                            sync_info=mybir.SyncInfo(on_wait=[w], on_update=[]),
                        ))
                    inst.sync_info = mybir.SyncInfo(
                        on_wait=[ow[-1]],
                        on_update=list(si.on_update) if si.on_update else [],
                    )
                new_list.append(inst)
            if changed:
                bb.instructions = new_list
    return nc


F32 = mybir.dt.float32
BF16 = mybir.dt.bfloat16
I16 = mybir.dt.int16

B, S, D, H, DH = 2, 2048, 1024, 16, 64
N_CORES = 8
HPC = H // (N_CORES // B)       # 4 heads per core
FPC = HPC * DH                  # 256 features per core
SQ = 512                        # query-block size (free dim of scores matmul)
SK = 128                        # key-block size (partition dim of scoresT)
NSQ = S // SQ                   # 4
NSK = S // SK                   # 16
KO = D // 128                   # 8 contraction blocks for the projections
EXP_SCALE = 1.0 / 8.0           # 1/sqrt(DH)

# DVE fast-exp (Schraudolph, bf16 bit trick): probs = bitcast_bf16(
# int16(score * 128/(ln2*8) + (127*128 + delta))). HW float->int16
# conversion is round-to-nearest (verified); delta = -4.5 centers the
# piecewise-linear 2^frac interpolation error (+-3.5% max, ~2% rms,
# systematic part cancels in the softmax normalization). Only a bounded
# fraction of tiles use this (error adds ~2% * sqrt(fraction) to output).
EXPA = 128.0 / (float(np.log(2.0)) * 8.0)
EXPB = 127.0 * 128.0 - 4.5

# schedule knobs (swept offline with TimelineSim)
CFG = dict(
    dve_exp_p1=0,    # sk2 tiles per pass-1 attn block exp'd on DVE (of 8)
    dve_exp_p2=0,    # ... per pass-2 attn block
    dve_exp_last=0,  # ... per attn block of the LAST sq heads 2/3 (ACT-paced
                     # end era with idle DVE; bounded accuracy cost)
    dve_exp_last01=0,  # ... last sq heads 0/1
    bs_act_p1=False,  # transpose-back/broadcast copy on ACT (else DVE)
    bs_act_p2=False,
    stage_act=0,     # out-proj staging copies routed to ACT (of 8 per sq)
    stage_act_last=4,  # ... additionally for the LAST sq only
    warm_first=False,  # emit PE warm-up before the load DMAs
    dma_variant=2,   # 0: csa/csb right after wq; 1: interleaved with x
                     # 2: wv right after csb; 3: wv between csa and csb
    rope_split0=True,  # split first k/q RoPE combines per head
    emit_variant=1,  # 0: two head-passes; 1: per-sq all-4-heads interleave
    qk_ahead_prio=-400000,   # priority offset for next-sq projections
    v_prio=-300000,          # priority offset for v projections (st 0-7)
    v_prio2=-1000000,        # priority offset for late v projections (8-15)
    out_prio=-2000000,       # priority offset for out-proj fill
    norm_prio=0,             # priority offset for the normalize chain
    psA_bufs=2,      # projection PSUM pool depth
    n_warm=20,       # PE warm-up dummy matmuls
    v_copy_act=0,    # v-proj PSUM->SBUF copies routed to ACT (of 16)
    tables_dve=False,  # replicate RoPE tables on DVE (4x) instead of ACT
    out_split_last=0,  # last-sq out-proj fo groups whose ko0 pre-accumulates
    prb_bufs=20,     # probs SBUF pool depth
    tmp_bufs=3,      # scratch SBUF pool depth
    ost_bufs=12,     # out-stage SBUF pool depth
    warm_tiny=False,  # 1-row warm-up operands (faster t=0 bootstrap)
    warm_nomemset=False,  # warm-up matmuls on uninitialized SBUF
    head_order=(0, 1, 2, 3),  # per-sq attention block order
    merge_at_bc=False,  # broadcast shares the at PSUM tile rows 64..127
    col_split_last=0,  # column-split normalize+out-proj of the last blocks
    table_rows=32,   # host-provided cos table rows (32, 64 or 128)
    table_rows_b=32,  # host-provided sin table rows (>= table_rows)
    wsplit_keep="last",  # which wait stays on the instruction (see _split)
    attn_pipe=False,  # software-pipelined attn emission order
    pv_nat=True,     # natural-layout PV + per-partition normalize + PE
                     # transpose back (output free size 65 vs 512)
    tp_psA=False,    # transpose PSUM tiles from the proj pool (less churn
                     # on the PV-accumulator pool)
    sk_group=2,      # key tiles per score-PSUM tile / exp instruction
    psS_bufs=2,      # score PSUM pool depth
    psAt_bufs=2,     # PV-accumulator PSUM pool depth
    dma_pairs=True,  # one output DMA per fo pair (halves HWDGE slots)
    tab_late_prio=0,  # deprioritize table replication rows 64-127
)


def _build():
    nc = bass.Bass()
    xT = nc.declare_dram_parameter("xT", [D, S], BF16, isOutput=False)
    wqp = nc.declare_dram_parameter("wq", [D, FPC], BF16, isOutput=False)
    wkp = nc.declare_dram_parameter("wk", [D, FPC], BF16, isOutput=False)
    wvp = nc.declare_dram_parameter("wv", [D, FPC], BF16, isOutput=False)
    wop = nc.declare_dram_parameter("wo", [FPC, D], BF16, isOutput=False)
    TR = CFG["table_rows"]
    TRB = max(TR, CFG["table_rows_b"])
    csap = nc.declare_dram_parameter("csa", [TR, S], BF16, isOutput=False)
    csbp = nc.declare_dram_parameter("csb", [TRB, S], BF16, isOutput=False)
    idp = nc.declare_dram_parameter("ident", [128, 128], BF16, isOutput=False)
    outp = nc.declare_dram_parameter("outT", [D, S], BF16, isOutput=True)

    with tile.TileContext(nc) as tc:
        with tc.tile_pool(name="persist", bufs=1) as pers, \
             tc.tile_pool(name="tmp", bufs=CFG["tmp_bufs"]) as tmp, \
             tc.tile_pool(name="probs", bufs=CFG["prb_bufs"]) as prb, \
             tc.tile_pool(name="ostage", bufs=CFG["ost_bufs"]) as ost, \
             tc.tile_pool(name="psA", bufs=CFG["psA_bufs"], space="PSUM") as psA, \
             tc.tile_pool(name="psS", bufs=CFG["psS_bufs"], space="PSUM") as psS, \
             tc.tile_pool(name="psAt", bufs=max(1, CFG["psAt_bufs"]), space="PSUM") as psAt:

            # ---------------- loads (all into dedicated tiles) -------------
            # order matters: the shared DMA device serializes transfers, so
            # the first qk-projection's inputs (wk + x chunk0, in ko-halves
            # so matmuls can start on the first half) go first; the RoPE
            # tables are only needed ~2 DMAs later.
            warm_in = pers.tile([128, 256], BF16, tag="warm")
            wps_pool = psA if CFG["psAt_bufs"] == 0 else psAt
            wps = wps_pool.tile([128, 256], F32,
                                tag="proj" if CFG["psAt_bufs"] == 0 else "attn",
                                name="warm_ps")

            def warmup():
                # PE warm-up: the HAM clock gate releases only after ~3.4us
                # of sustained PE activity; burn dummy matmuls on a zero tile
                # while the input DMAs are in flight so the real projections
                # run at 2.4 GHz from the start. Lowest priority: these fill
                # TensorE idle slots and keep the HAM activity window hot.
                if CFG["warm_nomemset"]:
                    # read the tile uninitialized: the product is never
                    # consumed (psum cleared by later start=True groups), and
                    # skipping the DVE memset lets PE activity - and the
                    # warm-clock ramp - start ~1.2us earlier
                    lhs, rhs = warm_in[:, 0:128], warm_in[:]
                elif CFG["warm_tiny"]:
                    with tc.high_priority():
                        nc.vector.memset(warm_in[0:1, :], 0.0)
                    lhs, rhs = warm_in[0:1, 0:128], warm_in[0:1, :]
                else:
                    nc.vector.memset(warm_in[:], 0.0)
                    lhs, rhs = warm_in[:, 0:128], warm_in[:]
                with tc.high_priority(offset=-3000000):
                    for _ in range(CFG["n_warm"]):
                        nc.tensor.matmul(wps[0:lhs.shape[1], :] if CFG["warm_tiny"] else wps[:],
                                         lhs, rhs, start=True, stop=True)
                nc.vector.memset(warm_in[0:1, 0:1], 0.0)

            if CFG["warm_first"]:
                warmup()

            xT_sb = pers.tile([128, KO, S], BF16, tag="xT")
            xTr = xT.rearrange("(ko p) s -> p ko s", p=128)
            wk_sb = pers.tile([128, KO, FPC], BF16, tag="wk")
            nc.sync.dma_start(wk_sb[:], wkp.rearrange("(ko p) m -> p ko m", p=128))

            def load_x(xc):
                for kh in range(2):
                    ks = bass.ts(kh, KO // 2)
                    nc.sync.dma_start(xT_sb[:, ks, bass.ts(xc, SQ)],
                                      xTr[:, ks, bass.ts(xc, SQ)])

            csa_sb = pers.tile([128, S], BF16, tag="csa")
            csb_sb = pers.tile([128, S], BF16, tag="csb")
            wv_sb = pers.tile([128, KO, FPC], BF16, tag="wv")
            wq_sb = pers.tile([128, KO, FPC], BF16, tag="wq")

            def load_wq():
                nc.sync.dma_start(wq_sb[:],
                                  wqp.rearrange("(ko p) m -> p ko m", p=128))

            if CFG["dma_variant"] == 4:
                # wq lands between the two x0 halves: the q projection's
                # first ko-half can start while k's second half still loads
                nc.sync.dma_start(xT_sb[:, 0:KO // 2, bass.ts(0, SQ)],
                                  xTr[:, 0:KO // 2, bass.ts(0, SQ)])
                load_wq()
                nc.sync.dma_start(xT_sb[:, KO // 2:KO, bass.ts(0, SQ)],
                                  xTr[:, KO // 2:KO, bass.ts(0, SQ)])
            else:
                load_x(0)
                load_wq()

            def load_tables():
                # the tables are 64-row periodic on-device ([cos;cos] and
                # [+s;-s]): DMA [TR, S] host-stacked rows and replicate the
                # rest with the otherwise-idle ScalarE (a scale=-1 copy
                # makes -s blocks when starting from [32, S]). ACT copy cost
                # depends on free size only, so fewer, taller copies win.
                nc.sync.dma_start(csa_sb[0:TR, :], csap[:])
                nc.sync.dma_start(csb_sb[0:TRB, :], csbp[:])
                CP = mybir.ActivationFunctionType.Copy
                # replication on the otherwise-idle ScalarE; csa/csb copies
                # INTERLEAVED (csa-r1, csb-r1, ...) so the first RoPE's
                # cos and +-sin rows both become available earliest.
                if TR == 32 and TRB == 32:
                    for r in range(1, 4):
                        sgn = -1.0 if r % 2 else 1.0
                        nc.scalar.activation(csa_sb[bass.ts(r, 32), :],
                                             csa_sb[0:32, :], CP)
                        nc.scalar.activation(csb_sb[bass.ts(r, 32), :],
                                             csb_sb[0:32, :], CP, scale=sgn)
                else:
                    if TR == 32:
                        for r in range(1, 4):
                            nc.scalar.activation(csa_sb[bass.ts(r, 32), :],
                                                 csa_sb[0:32, :], CP)
                    elif TR == 64:
                        nc.scalar.activation(csa_sb[64:128, :],
                                             csa_sb[0:64, :], CP)
                    if TRB == 32:
                        for r in range(1, 4):
                            sgn = -1.0 if r % 2 else 1.0
                            nc.scalar.activation(csb_sb[bass.ts(r, 32), :],
                                                 csb_sb[0:32, :], CP,
                                                 scale=sgn)
                    elif TRB == 64:
                        nc.scalar.activation(csb_sb[64:128, :],
                                             csb_sb[0:64, :], CP)

            def load_wv():
                nc.sync.dma_start(wv_sb[:],
                                  wvp.rearrange("(ko p) m -> p ko m", p=128))

            v = CFG["dma_variant"]
            if v == 0:
                load_tables()
                for xc in range(1, NSQ):
                    load_x(xc)
                load_wv()
            elif v == 1:
                load_x(1)
                load_tables()
                load_x(2)
                load_x(3)
                load_wv()
            elif v in (2, 4):
                load_tables()
                load_wv()
                for xc in range(1, NSQ):
                    load_x(xc)
            else:
                load_tables()
                load_wv()
                for xc in range(1, NSQ):
                    load_x(xc)
            wo_sb = pers.tile([128, FPC // 128, D], BF16, tag="wo")
            nc.sync.dma_start(wo_sb[:], wop.rearrange("(ko p) m -> p ko m", p=128))
            id_sb = pers.tile([128, 128], BF16, tag="ident")
            if CFG["pv_nat"]:
                nc.sync.dma_start(id_sb[:], idp[:])

            if not CFG["warm_first"]:
                warmup()

            # ones column for the denominator broadcast matmul
            ones_sb = pers.tile([1, DH], BF16, tag="ones")
            nc.vector.memset(ones_sb[:], 1.0)

            # persistent activations
            qT = [pers.tile([128, S], BF16, tag=f"qT{ft}", name=f"qT{ft}")
                  for ft in range(2)]
            kT = [pers.tile([128, S], BF16, tag=f"kT{ft}", name=f"kT{ft}")
                  for ft in range(2)]
            # [v | 1] as PV stationary tiles: per (sk, head) a [128, DH+1]
            v_sb = pers.tile([128, NSK, HPC, DH + 1], BF16, tag="v")
            nc.vector.memset(v_sb[:, :, :, DH:], 1.0)
            # attention output (bf16, feeds the out-projection)
            aT = [pers.tile([128, S], BF16, tag=f"aT{ft}", name=f"aT{ft}")
                  for ft in range(2)]

            # ---------------- v projection (natural layout) ---------------
            def v_proj(st):
                ps = psA.tile([128, FPC], F32, tag="proj", name="vproj_ps")
                for ko in range(KO):
                    nc.tensor.matmul(
                        ps[:],
                        xT_sb[:, ko, bass.ts(st, 128)],
                        wv_sb[:, ko, :],
                        start=(ko == 0), stop=(ko == KO - 1),
                    )
                if st < CFG["v_copy_act"]:
                    # ScalarE is idle during the start window; keeping these
                    # copies off DVE (busy with RoPE) frees psA slots sooner
                    nc.scalar.copy(
                        v_sb[:, st, :, 0:DH],
                        ps.rearrange("p (h d) -> p h d", h=HPC))
                else:
                    nc.vector.tensor_copy(
                        v_sb[:, st, :, 0:DH],
                        ps.rearrange("p (h d) -> p h d", h=HPC))

            # ---------------- q/k projection + RoPE ------------------------
            # psum rows per head offset: [t0 (32) ; t1 (32)]. One PSUM->SBUF
            # bf16 copy, then 6 SBUF ops at the DVE 2x rate:
            #   mc       = qs * cos_rep                       (128 rows)
            #   msx[ 0:32 ] = qs[32:64 ] * csb[32:64 ]  (= -t1*s: csb row
            #   msx[32:64 ] = qs[ 0:32 ] * csb[ 0:32 ]   blocks alternate
            #   msx[64:96 ] = qs[96:128] * csb[96:128]   +s,-s,+s,-s so all
            #   msx[96:128] = qs[64:96 ] * csb[64:96 ]   combines are ADDs)
            #   dst      = mc + msx                           (128 rows)
            # (partition patterns at base 32/96 are limited to 32 partitions
            # on this walrus, hence the 32-aligned strips; both SBUF inputs
            # of a TensorTensor must share their base partition, the output
            # may differ)
            def qk_proj(w_sb, dst, ft, sq, split_heads=False):
                sl = bass.ts(sq, SQ)
                ps = psA.tile([128, SQ], F32, tag="proj", name="qkproj_ps")
                for ko in range(KO):
                    nc.tensor.matmul(
                        ps[:],
                        w_sb[:, ko, bass.ts(ft, 128)],
                        xT_sb[:, ko, bass.ts(sq, SQ)],
                        start=(ko == 0), stop=(ko == KO - 1),
                    )
                qs = tmp.tile([128, SQ], BF16, tag="ropeQS")
                nc.vector.tensor_copy(qs[:], ps[:])
                mc = tmp.tile([128, SQ], BF16, tag="ropeMC")
                msx = tmp.tile([128, SQ], BF16, tag="ropeMSX")
                if split_heads:
                    # per-head chains so the first head's scores can issue
                    # before the second head's RoPE finishes (start latency)
                    nc.vector.tensor_mul(mc[0:64, :], qs[0:64, :],
                                         csa_sb[0:64, sl])
                    nc.vector.tensor_mul(msx[0:32, :], qs[32:64, :],
                                         csb_sb[32:64, sl])
                    nc.vector.tensor_mul(msx[32:64, :], qs[0:32, :],
                                         csb_sb[0:32, sl])
                    nc.vector.tensor_add(dst[0:64, sl], mc[0:64, :],
                                         msx[0:64, :])
                    nc.vector.tensor_mul(mc[64:128, :], qs[64:128, :],
                                         csa_sb[64:128, sl])
                    nc.vector.tensor_mul(msx[64:96, :], qs[96:128, :],
                                         csb_sb[96:128, sl])
                    nc.vector.tensor_mul(msx[96:128, :], qs[64:96, :],
                                         csb_sb[64:96, sl])
                    nc.vector.tensor_add(dst[64:128, sl], mc[64:128, :],
                                         msx[64:128, :])
                    return
                nc.vector.tensor_mul(mc[:], qs[:], csa_sb[:, sl])
                nc.vector.tensor_mul(msx[0:32, :], qs[32:64, :],
                                     csb_sb[32:64, sl])
                nc.vector.tensor_mul(msx[32:64, :], qs[0:32, :],
                                     csb_sb[0:32, sl])
                nc.vector.tensor_mul(msx[64:96, :], qs[96:128, :],
                                     csb_sb[96:128, sl])
                nc.vector.tensor_mul(msx[96:128, :], qs[64:96, :],
                                     csb_sb[64:96, sl])
                nc.vector.tensor_add(dst[:, sl], mc[:], msx[:])

            # ---------------- attention block ------------------------------
            def attn_block(sq, h, n_dve_exp=0, bs_act=True, col_split=False):
                sl = bass.ts(sq, SQ)
                ft, off = h // 2, (h % 2) * 64
                if CFG["pv_nat"]:
                    at = None   # natural-PV path allocates its own psum
                elif CFG["merge_at_bc"]:
                    # one 128-partition tile per block: PV accumulates into
                    # rows 0..64 and the ones-broadcast matmul reuses rows
                    # 64..127 (the reciprocal reads the denominator row
                    # before the broadcast overwrites it). Keeps bc from
                    # occupying a second psAt slot, so block n+1's PV can
                    # start while block n's normalize still runs.
                    at = psAt.tile([128, SQ], F32, tag="attn")
                else:
                    at = psAt.tile([DH + 1, SQ], F32, tag="attn")
                # spread the DVE-exp'd tiles across the block
                dve_tiles = {NSK // 2 - 1 - 2 * j for j in range(n_dve_exp)}
                def emit_sc_exp_g(sks, dve):
                    # one score tile + ONE exp instruction for a GROUP of
                    # key tiles (bigger groups amortize the per-exp access
                    # overhead and slot-recycle pitch on ScalarE)
                    g = len(sks)
                    GW = CFG["sk_group"]
                    sc = psS.tile([128, GW, SQ], F32, tag="sc")
                    pb = prb.tile([128, GW, SQ], BF16, tag="pb")
                    for i, sk in enumerate(sks):
                        nc.tensor.matmul(
                            sc[:, i, :],
                            kT[ft][off:off + 64, bass.ts(sk, SK)],
                            qT[ft][off:off + 64, sl],
                            start=True, stop=True,
                        )
                    if dve:
                        with nc.allow_low_precision(reason="fast exp"):
                            nc.vector.tensor_scalar(
                                pb[:, 0:g, :].bitcast(I16), sc[:, 0:g, :],
                                EXPA, EXPB,
                                mybir.AluOpType.mult, mybir.AluOpType.add)
                    else:
                        nc.scalar.activation(
                            pb[:, 0:g, :], sc[:, 0:g, :],
                            mybir.ActivationFunctionType.Exp, scale=EXP_SCALE)
                    return pb

                def emit_sc_exp(sk2):
                    return emit_sc_exp_g([2 * sk2, 2 * sk2 + 1],
                                         sk2 in dve_tiles)

                def emit_pv(sk2, pb):
                    for i in range(2):
                        sk = 2 * sk2 + i
                        nc.tensor.matmul(
                            at[0:DH + 1, :], v_sb[:, sk, h, :], pb[:, i, :],
                            start=(sk == 0), stop=(sk == NSK - 1),
                        )

                def emit_pv_nat(sk2, pb, atn):
                    # natural-layout PV: probs is the STATIONARY operand so
                    # the output is [128 queries, DH+1] -- free size 65
                    # instead of 512, 4x cheaper on TensorE per element.
                    # PSUM start=True zeroes the WHOLE 2KB bank
                    # (ZERO_REGION_SIZE), so only the very first matmul may
                    # carry it: the other query-subtiles' first writes
                    # accumulate onto the already-zeroed bank.
                    for i in range(2):
                        sk = 2 * sk2 + i
                        for qs4 in range(4):
                            nc.tensor.matmul(
                                atn[:, qs4, :],
                                pb[:, i, bass.ts(qs4, 128)],
                                v_sb[:, sk, h, :],
                                start=(sk == 0 and qs4 == 0),
                                stop=(sk == NSK - 1),
                                skip_group_check=True,
                            )

                if CFG["pv_nat"]:
                    atn_pool = psA if CFG["psAt_bufs"] == 0 else psAt
                    atn = atn_pool.tile(
                        [128, 4, DH + 1], F32,
                        tag="proj" if CFG["psAt_bufs"] == 0 else "attn",
                        name="at_nat")
                    GW = CFG["sk_group"]
                    groups = [list(range(s, min(s + GW, NSK)))
                              for s in range(0, NSK, GW)]
                    for gi, sks in enumerate(groups):
                        pbs = emit_sc_exp_g(sks, False)
                        for i, sk in enumerate(sks):
                            for qs4 in range(4):
                                nc.tensor.matmul(
                                    atn[:, qs4, :],
                                    pbs[:, i, bass.ts(qs4, 128)],
                                    v_sb[:, sk, h, :],
                                    start=(sk == 0 and qs4 == 0),
                                    stop=(sk == NSK - 1),
                                    skip_group_check=True,
                                )
                    for qs4 in range(4):
                        # per-partition normalize (queries on partitions):
                        # no broadcast needed at all
                        rcn = tmp.tile([128, 1], F32, tag="recip", name="rcn")
                        with nc.allow_low_precision(
                                reason="softmax denominator"):
                            nc.vector.reciprocal(
                                rcn[:], atn[:, qs4, DH:DH + 1])
                        ann = tmp.tile([128, DH], BF16, tag="anat",
                                       name="ann")
                        nc.vector.tensor_scalar(
                            ann[:], atn[:, qs4, 0:DH], rcn[:], None,
                            mybir.AluOpType.mult)
                        # transpose back to [features, queries] for the
                        # out-projection (PE transpose mode, bf16)
                        use_psA = CFG["tp_psA"] or CFG["psAt_bufs"] == 0
                        tp = (psA if use_psA else psAt).tile(
                            [DH, 128], BF16,
                            tag="proj" if use_psA else "attn",
                            name="tp_ps")
                        nc.tensor.transpose(tp[:], ann[:], id_sb[:])
                        csl = bass.ts(4 * sq + qs4, 128)
                        if bs_act:
                            nc.scalar.copy(aT[ft][off:off + 64, csl], tp[:])
                        else:
                            nc.vector.tensor_copy(aT[ft][off:off + 64, csl],
                                                  tp[:])
                    return
                if CFG["attn_pipe"]:
                    # software-pipelined emission: next tile's scores sit
                    # ahead of this tile's PV in the tie-break order
                    pbs = emit_sc_exp(0)
                    for sk2 in range(1, NSK // 2):
                        pb_next = emit_sc_exp(sk2)
                        emit_pv(sk2 - 1, pbs)
                        pbs = pb_next
                    emit_pv(NSK // 2 - 1, pbs)
                else:
                    for sk2 in range(NSK // 2):
                        pbs = emit_sc_exp(sk2)
                        emit_pv(sk2, pbs)
                ctx = tc.high_priority(offset=CFG["norm_prio"]) \
                    if CFG["norm_prio"] else None
                if ctx is not None:
                    ctx.__enter__()
                # col_split: run the normalize per column half so the first
                # half of the (column-split) out-projection can start while
                # the second half still normalizes -- shortens the epilogue
                # of the final attention block.
                SH = SQ // 2
                halves = ((0, SH), (SH, SH)) if col_split else ((0, SQ),)
                for c0, cw in halves:
                    cs = slice(c0, c0 + cw)
                    sls = bass.ts(2 * sq + c0 // SH, SH) if col_split else sl
                    rc = tmp.tile([1, cw], BF16, tag="recip", name="rc")
                    with nc.allow_low_precision(reason="softmax denominator"):
                        nc.vector.reciprocal(rc[:], at[DH:DH + 1, cs])
                    if CFG["merge_at_bc"]:
                        bc = at[DH:2 * DH, cs]
                        nc.tensor.matmul(bc, ones_sb[:], rc[:],
                                         start=True, stop=True,
                                         skip_group_check=True)
                    else:
                        bct = psAt.tile([DH, cw], F32, tag="attn",
                                        name="bcast_ps")
                        bc = bct[:]
                        nc.tensor.matmul(bc, ones_sb[:], rc[:],
                                         start=True, stop=True)
                    bs = tmp.tile([DH, cw], F32, tag="bcsb", name="bs")
                    if bs_act:
                        nc.scalar.copy(bs[:], bc)
                    else:
                        nc.vector.tensor_copy(bs[:], bc)
                    nc.vector.tensor_mul(aT[ft][off:off + 64, sls],
                                         at[0:DH, cs], bs[:])
                if ctx is not None:
                    ctx.__exit__(None, None, None)

            # ---------------- out-projection for one query block -----------
            outpR = outp.rearrange("(fo p) s -> p fo s", p=128)

            def out_proj_finish(sq, fo, po, stg=None):
                sl = bass.ts(sq, SQ)
                on_act = fo < CFG["stage_act"]
                if sq == NSQ - 1 and fo % 2 == 1 and \
                        fo < 2 * CFG["stage_act_last"]:
                    on_act = True
                if stg is None:
                    stg1 = ost.tile([128, SQ], BF16, tag="oT", name="stg1")
                    dst = stg1[:]
                else:
                    stg1 = None
                    dst = stg
                if on_act:
                    nc.scalar.copy(dst, po[:])
                else:
                    nc.vector.tensor_copy(dst, po[:])
                if stg1 is not None:
                    nc.sync.dma_start(outp[bass.ts(fo, 128), sl], dst)

            def out_proj_ko(sq, fo, po, ko, start, stop):
                nc.tensor.matmul(
                    po[:],
                    wo_sb[:, ko, bass.ts(fo, 128)],
                    aT[ko][:, bass.ts(sq, SQ)],
                    start=start, stop=stop,
                )

            def out_proj(sq, skip_fo=()):
                last = sq == NSQ - 1
                pair = CFG["dma_pairs"]
                csplit = last and CFG["col_split_last"]
                sl = bass.ts(sq, SQ)
                SH = SQ // 2
                stg2 = None
                for fo in range(8):
                    if fo in skip_fo:
                        continue
                    # on the last block the scores stream is done, so its
                    # PSUM pool is free: borrow it for 2 extra po slots
                    if last and fo % 2 == 1:
                        po = psS.tile([128, SQ], F32, tag="sc", name="oproj_ps2")
                    else:
                        po = psA.tile([128, SQ], F32, tag="proj", name="oproj_ps")
                    if csplit:
                        # column-split: the first half contracts aT columns
                        # that finish normalizing earlier
                        for ch in range(2):
                            ccs = slice(ch * SH, (ch + 1) * SH)
                            for ko in range(2):
                                nc.tensor.matmul(
                                    po[:, ccs],
                                    wo_sb[:, ko, bass.ts(fo, 128)],
                                    aT[ko][:, bass.ts(2 * sq + ch, SH)],
                                    start=(ko == 0), stop=(ko == 1),
                                )
                    else:
                        out_proj_ko(sq, fo, po, 0, True, False)
                        out_proj_ko(sq, fo, po, 1, False, True)
                    if not pair:
                        out_proj_finish(sq, fo, po)
                        continue
                    # paired staging: two fo blocks share one [128,2,SQ]
                    # tile and ONE output DMA (halves the serial HWDGE
                    # descriptor-generation slots that pace the tail)
                    if fo % 2 == 0:
                        stg2 = ost.tile([128, 2, SQ], BF16, tag="oT")
                        dsts = stg2[:, 0, :]
                    else:
                        dsts = stg2[:, 1, :]
                    if csplit:
                        # stage per column half (alternating engines) so
                        # the first half's copy runs during the second
                        # half's matmuls
                        for ch in range(2):
                            ccs = slice(ch * SH, (ch + 1) * SH)
                            if (fo + ch) % 2 == 0:
                                nc.vector.tensor_copy(dsts[:, ccs],
                                                      po[:, ccs])
                            else:
                                nc.scalar.copy(dsts[:, ccs], po[:, ccs])
                    else:
                        out_proj_finish(sq, fo, po, stg=dsts)
                    if fo % 2 == 1:
                        nc.sync.dma_start(outpR[:, fo - 1:fo + 1, sl],
                                          stg2[:])

            # ---------------- emission order (overlap) ---------------------
            if CFG["emit_variant"] == 0:
                # two head-passes: heads 0,1 for all sq, then 2,3 + out-proj
                qk_proj(wk_sb, kT[0], 0, 0, split_heads=CFG["rope_split0"])
                qk_proj(wq_sb, qT[0], 0, 0, split_heads=CFG["rope_split0"])
                for sq in range(1, NSQ):
                    qk_proj(wk_sb, kT[0], 0, sq)
                with tc.high_priority(offset=-400000):
                    for sq in range(1, NSQ):
                        qk_proj(wq_sb, qT[0], 0, sq)
                with tc.high_priority(offset=CFG["v_prio"]):
                    for st in range(8):
                        v_proj(st)
                with tc.high_priority(offset=-1000000):
                    for st in range(8, NSK):
                        v_proj(st)
                for sq in range(NSQ):
                    attn_block(sq, 0, CFG["dve_exp_p1"], CFG["bs_act_p1"])
                    attn_block(sq, 1, CFG["dve_exp_p1"], CFG["bs_act_p1"])
                with tc.high_priority(offset=-500000):
                    for sq in range(NSQ):
                        qk_proj(wk_sb, kT[1], 1, sq)
                    for sq in range(NSQ):
                        qk_proj(wq_sb, qT[1], 1, sq)
                for sq in range(NSQ):
                    attn_block(sq, 2, CFG["dve_exp_p2"], CFG["bs_act_p2"])
                    attn_block(sq, 3, CFG["dve_exp_p2"], CFG["bs_act_p2"])
                    with tc.high_priority(offset=CFG["out_prio"]):
                        out_proj(sq)
            else:
                # per-sq: all 4 heads of each query block back-to-back, with
                # the next block's projections + v + out-proj as PE fill --
                # balances the ACT exp stream across the whole kernel span.
                qk_proj(wk_sb, kT[0], 0, 0, split_heads=CFG["rope_split0"])
                qk_proj(wq_sb, qT[0], 0, 0, split_heads=CFG["rope_split0"])
                with tc.high_priority(offset=CFG["qk_ahead_prio"]):
                    qk_proj(wk_sb, kT[1], 1, 0)
                    qk_proj(wq_sb, qT[1], 1, 0)
                with tc.high_priority(offset=CFG["v_prio"]):
                    for st in range(8):
                        v_proj(st)
                with tc.high_priority(offset=-1000000):
                    for st in range(8, NSK):
                        v_proj(st)
                ho = CFG["head_order"]
                for sq in range(NSQ):
                    last = sq == NSQ - 1
                    nsplit = CFG["out_split_last"] if last else 0
                    de1 = CFG["dve_exp_last01"] if last else CFG["dve_exp_p1"]
                    attn_block(sq, ho[0], de1, CFG["bs_act_p1"])
                    attn_block(sq, ho[1], de1, CFG["bs_act_p1"])
                    if sq + 1 < NSQ:
                        with tc.high_priority(offset=CFG["qk_ahead_prio"]):
                            qk_proj(wk_sb, kT[0], 0, sq + 1)
                            qk_proj(wq_sb, qT[0], 0, sq + 1)
                    # last sq: psA is otherwise idle now, so pre-accumulate
                    # the ko0 half (reads aT[0] = heads 0,1, already final)
                    # of the first fo groups; only ko1+stage+DMA remain
                    # after the last head's normalize.
                    pre = []
                    for fo in range(nsplit):
                        po = psA.tile([128, SQ], F32, tag="proj",
                                      name="oproj_ps")
                        out_proj_ko(sq, fo, po, 0, True, False)
                        pre.append((fo, po))
                    de2 = CFG["dve_exp_last"] if last else CFG["dve_exp_p2"]
                    ncs = CFG["col_split_last"] if last else 0
                    attn_block(sq, ho[2], de2, CFG["bs_act_p2"],
                               col_split=(ncs >= 2))
                    attn_block(sq, ho[3], de2, CFG["bs_act_p2"],
                               col_split=(ncs >= 1))
                    if sq + 1 < NSQ:
                        with tc.high_priority(offset=CFG["qk_ahead_prio"]):
                            qk_proj(wk_sb, kT[1], 1, sq + 1)
                            qk_proj(wq_sb, qT[1], 1, sq + 1)
                    with tc.high_priority(offset=CFG["out_prio"]):
                        for fo, po in pre:
                            out_proj_ko(sq, fo, po, 1, False, True)
                            out_proj_finish(sq, fo, po)
                        out_proj(sq, skip_fo=tuple(f for f, _ in pre))

    _split_multi_waits(nc, keep=CFG["wsplit_keep"])
    return nc


_NC_CACHE = None


def _get_nc():
    global _NC_CACHE
    if _NC_CACHE is None:
        _NC_CACHE = _build()
    return _NC_CACHE


# rotation-pair permutation: within each head, [0,2,...,62, 1,3,...,63]
_PAIR_PERM = np.concatenate([np.arange(0, DH, 2), np.arange(1, DH, 2)])


def kernel(x, freqs_cos, freqs_sin, wq, wk, wv, wo):
    x = np.asarray(x, dtype=np.float32)
    cosT = np.asarray(freqs_cos, np.float32).T    # [32, S]
    sinT = np.asarray(freqs_sin, np.float32).T
    # host-stacked base tables (the kernel replicates the rest on-device;
    # the on-device pattern is [cos]x4 and [+s,-s,+s,-s] per 32-row block)
    TR = CFG["table_rows"]
    TRB = max(TR, CFG["table_rows_b"])
    csa = np.ascontiguousarray(
        np.concatenate([cosT] * (TR // 32), 0)).astype(ml_dtypes.bfloat16)
    sgn = [sinT if r % 2 == 0 else -sinT for r in range(TRB // 32)]
    csb = np.ascontiguousarray(np.concatenate(sgn, 0)).astype(
        ml_dtypes.bfloat16)
    wq = np.asarray(wq, np.float32)
    wk = np.asarray(wk, np.float32)
    wv = np.asarray(wv, np.float32)
    wo = np.asarray(wo, np.float32)

    bf = ml_dtypes.bfloat16
    in_maps = []
    for c in range(N_CORES):
        b, hg = divmod(c, N_CORES // B)
        heads = [hg * HPC + i for i in range(HPC)]
        qk_cols = np.concatenate([h * DH + _PAIR_PERM for h in heads])
        v_cols = np.concatenate([h * DH + np.arange(DH) for h in heads])
        in_maps.append({
            "ident": np.eye(128, dtype=np.float32).astype(bf),
            "xT": np.ascontiguousarray(x[b].T).astype(bf),
            "wq": np.ascontiguousarray(wq[:, qk_cols]).astype(bf),
            "wk": np.ascontiguousarray(wk[:, qk_cols]).astype(bf),
            "wv": np.ascontiguousarray(wv[:, v_cols]).astype(bf),
            "wo": np.ascontiguousarray(wo[v_cols, :]).astype(bf),
            "csa": csa,
            "csb": csb,
        })

    nc = _get_nc()
    res = run_bass_kernel_spmd(nc, in_maps, core_ids=list(range(N_CORES)))

    out = np.zeros((B, S, D), dtype=np.float32)
    for c in range(N_CORES):
        b = c // (N_CORES // B)
        out[b] += res.results[c]["outT"].astype(np.float32).T
    return out



# revision 89
# speedup vs baseline: 1.0084x; 1.0013x over previous
"""Multi-head attention (RoPE) forward for Trainium2, 8 NeuronCores.

Problem: B=2, S=2048, D=1024, H=16 heads, Dh=64, fp32 in/out.

Sharding (host side): data-parallel over the 2 batches x 4-way tensor
parallel over heads -> each of the 8 cores handles (batch b, 4 heads) with
its column slice of wq/wk/wv and row slice of wo. Each core returns a
partial output out[b].T contribution; the host sums the 4 partials per
batch (the wo row-reduction).

Device kernel (per core), all in "transposed" layout (features on SBUF
partitions, sequence on the free dim) so no on-device transposes are
needed (the host feeds x[b].T):

  qT = (wq_c)^T x^T, kT likewise (PSUM fp32, bf16 operands)
  RoPE via DVE, all in SBUF bf16 (2x mode). The host pre-permutes wq/wk
      columns so rotation pair elements land at partitions j and j+32
      (contiguous blocks; the permutation cancels in q.k) and supplies
      32-row-replicated cos tables plus a SIGN-ALTERNATING sin table
      (+s,-s,+s,-s per 32-row block). Per 512-col block this takes 7 DVE
      ops: qs copy, mc = qs*cos, 4 partition-shifted msx strips (the +-
      signs baked into the table make every combine an ADD), and ONE
      full-128-row combine qT = mc + msx. (A both-SBUF TensorTensor must
      share base partition on this walrus; non-{0,64} bases max 32 rows.)
  v  = x wv_c in natural [S, 256] layout (x^T used as lhsT)
  per (head, 512-query block): for each pair of 128-key blocks:
      scoresT = kT_tile^T qT_block (K=64 contraction, one PSUM bank each)
      probsT  = exp(scoresT / 8)  (ScalarE, 1024-wide straight from PSUM)
      attn^T += [v_tile | 1]^T probsT   (ones column yields the softmax
                                         denominator as attn^T row 64)
  normalize: recip = 1/denominator (DVE); broadcast across 64 partitions
      via a rank-1 ones matmul (PE); PSUM->SBUF copy (ACT for heads 0/1,
      DVE for 2/3 -- balance found by TimelineSim sweep); multiply (DVE)
  outT = wo_c^T attn_out^T (accumulated over the 2 K-blocks); PSUM ->
      bf16 staging -> DMA out (host accumulates partials in fp32).
      Staging tiles are PAIRED ([128,2,512], one DMA per fo pair) because
      the kernel tail is paced by the serial per-transfer HWDGE
      descriptor-generation slots, not by the copies; for the last query
      block the two halves of each pair are staged on different engines
      (DVE/ACT) so a pair completes in one copy-time.

  The RoPE tables are DMA'd as [32, S] and replicated on-device by the
  otherwise-idle ScalarE (scale=-1 copies make the -s blocks), keeping
  the serial input-DMA stream short: wk, x0, wq, tables, wv, x1-3, wo,
  so TensorE's first projections and the v-projection fill the
  DMA-starved start window. (GpSimd extended-ISA ops - partition
  broadcast/reduce - do not compile on this walrus; plain Pool
  TensorTensor compiles but returns garbage on HW, so Pool is unusable
  for compute and everything balances across PE/ACT/DVE.)

The walrus build here accepts only ONE sync wait per instruction; Tile
emits more. _split_multi_waits legalizes the final BIR by hoisting extra
waits onto same-engine NoOps (identical semantics: waits execute on the
engine sequencer in program order).
"""
import sys

for _p in ("/opt/trn_rl_repo",):
    if _p not in sys.path:
        sys.path.insert(0, _p)

import numpy as np
import ml_dtypes

import concourse.bass as bass
import concourse.mybir as mybir
import concourse.tile as tile
import concourse.tile_sem_assignment as _tsa

# 3 engine sems + 4 DMA queues (re-tuned after the natural-PV
# restructure: 4 and 8 tie at best, 6 is +200ns).
_tsa.NUM_HWDGE_SEMS = 4

from concourse.bass_utils import run_bass_kernel_spmd

_wsplit_ctr = [0]


def _split_multi_waits(nc, keep="last"):
    """Legalize the BIR for this walrus build (max ONE sync wait per
    instruction): hoist all but one wait of any instruction onto
    same-engine NoOps placed directly before it. Waits execute on the
    engine's sequencer in program order, so this is semantics-preserving.
    keep: which wait stays on the real instruction ("last" or "first") --
    the NoOps' waits block the SEQ while the instruction's own wait parks
    in the non-blocking wait queue, so the choice shifts head-of-line
    blocking."""
    for f in nc.m.functions:
        for bb in f.blocks:
            insts = bb.instructions
            new_list = []
            changed = False
            for inst in insts:
                si = inst.sync_info
                ow = list(si.on_wait) if (si is not None and si.on_wait) else []
                if len(ow) > 1:
                    changed = True
                    if keep == "first":
                        ow = [ow[0]] + ow[1:][::-1]
                        ow = ow[1:] + ow[:1]
                    for w in ow[:-1]:
                        _wsplit_ctr[0] += 1
                        new_list.append(mybir.InstNoOp(
                            name=f"I-wsplit-{_wsplit_ctr[0]}",
                            engine=inst.engine,
                            ins=[], outs=[],
                            sync_info=mybir.SyncInfo(on_wait=[w], on_update=[]),
                        ))
                    inst.sync_info = mybir.SyncInfo(
                        on_wait=[ow[-1]],
                        on_update=list(si.on_update) if si.on_update else [],
                    )
                new_list.append(inst)
            if changed:
                bb.instructions = new_list
    return nc


F32 = mybir.dt.float32
BF16 = mybir.dt.bfloat16
I16 = mybir.dt.int16

B, S, D, H, DH = 2, 2048, 1024, 16, 64
N_CORES = 8
HPC = H // (N_CORES // B)       # 4 heads per core
FPC = HPC * DH                  # 256 features per core
SQ = 512                        # query-block size (free dim of scores matmul)
SK = 128                        # key-block size (partition dim of scoresT)
NSQ = S // SQ                   # 4
NSK = S // SK                   # 16
KO = D // 128                   # 8 contraction blocks for the projections
EXP_SCALE = 1.0 / 8.0           # 1/sqrt(DH)

# DVE fast-exp (Schraudolph, bf16 bit trick): probs = bitcast_bf16(
# int16(score * 128/(ln2*8) + (127*128 + delta))). HW float->int16
# conversion is round-to-nearest (verified); delta = -4.5 centers the
# piecewise-linear 2^frac interpolation error (+-3.5% max, ~2% rms,
# systematic part cancels in the softmax normalization). Only a bounded
# fraction of tiles use this (error adds ~2% * sqrt(fraction) to output).
EXPA = 128.0 / (float(np.log(2.0)) * 8.0)
EXPB = 127.0 * 128.0 - 4.5

# schedule knobs (swept offline with TimelineSim)
CFG = dict(
    dve_exp_p1=0,    # sk2 tiles per pass-1 attn block exp'd on DVE (of 8)
    dve_exp_p2=0,    # ... per pass-2 attn block
    dve_exp_last=0,  # ... per attn block of the LAST sq heads 2/3 (ACT-paced
                     # end era with idle DVE; bounded accuracy cost)
    dve_exp_last01=0,  # ... last sq heads 0/1
    bs_act_p1=False,  # transpose-back/broadcast copy on ACT (else DVE)
    bs_act_p2=False,
    stage_act=0,     # out-proj staging copies routed to ACT (of 8 per sq)
    stage_act_last=4,  # ... additionally for the LAST sq only
    warm_first=False,  # emit PE warm-up before the load DMAs
    dma_variant=2,   # 0: csa/csb right after wq; 1: interleaved with x
                     # 2: wv right after csb; 3: wv between csa and csb
    rope_split0=True,  # split first k/q RoPE combines per head
    emit_variant=1,  # 0: two head-passes; 1: per-sq all-4-heads interleave
    qk_ahead_prio=-400000,   # priority offset for next-sq projections
    v_prio=-300000,          # priority offset for v projections (st 0-7)
    v_prio2=-1000000,        # priority offset for late v projections (8-15)
    out_prio=-2000000,       # priority offset for out-proj fill
    norm_prio=0,             # priority offset for the normalize chain
    psA_bufs=2,      # projection PSUM pool depth
    n_warm=20,       # PE warm-up dummy matmuls
    v_copy_act=0,    # v-proj PSUM->SBUF copies routed to ACT (of 16)
    tables_dve=False,  # replicate RoPE tables on DVE (4x) instead of ACT
    out_split_last=0,  # last-sq out-proj fo groups whose ko0 pre-accumulates
    prb_bufs=20,     # probs SBUF pool depth
    tmp_bufs=3,      # scratch SBUF pool depth
    ost_bufs=16,     # out-stage SBUF pool depth
    warm_tiny=False,  # 1-row warm-up operands (faster t=0 bootstrap)
    warm_nomemset=False,  # warm-up matmuls on uninitialized SBUF
    head_order=(0, 1, 2, 3),  # per-sq attention block order
    merge_at_bc=False,  # broadcast shares the at PSUM tile rows 64..127
    col_split_last=0,  # column-split normalize+out-proj of the last blocks
    table_rows=32,   # host-provided cos table rows (32, 64 or 128)
    table_rows_b=32,  # host-provided sin table rows (>= table_rows)
    wsplit_keep="last",  # which wait stays on the instruction (see _split)
    attn_pipe=False,  # software-pipelined attn emission order
    pv_nat=True,     # natural-layout PV + per-partition normalize + PE
                     # transpose back (output free size 65 vs 512)
    tp_psA=False,    # transpose PSUM tiles from the proj pool (less churn
                     # on the PV-accumulator pool)
    sk_group=2,      # key tiles per score-PSUM tile / exp instruction
    psS_bufs=2,      # score PSUM pool depth
    psAt_bufs=2,     # PV-accumulator PSUM pool depth
    dma_pairs=True,  # one output DMA per fo pair (halves HWDGE slots)
    tab_late_prio=0,  # deprioritize table replication rows 64-127
)


def _build():
    nc = bass.Bass()
    xT = nc.declare_dram_parameter("xT", [D, S], BF16, isOutput=False)
    wqp = nc.declare_dram_parameter("wq", [D, FPC], BF16, isOutput=False)
    wkp = nc.declare_dram_parameter("wk", [D, FPC], BF16, isOutput=False)
    wvp = nc.declare_dram_parameter("wv", [D, FPC], BF16, isOutput=False)
    wop = nc.declare_dram_parameter("wo", [FPC, D], BF16, isOutput=False)
    TR = CFG["table_rows"]
    TRB = max(TR, CFG["table_rows_b"])
    csap = nc.declare_dram_parameter("csa", [TR, S], BF16, isOutput=False)
    csbp = nc.declare_dram_parameter("csb", [TRB, S], BF16, isOutput=False)
    idp = nc.declare_dram_parameter("ident", [128, 128], BF16, isOutput=False)
    outp = nc.declare_dram_parameter("outT", [D, S], BF16, isOutput=True)

    with tile.TileContext(nc) as tc:
        with tc.tile_pool(name="persist", bufs=1) as pers, \
             tc.tile_pool(name="tmp", bufs=CFG["tmp_bufs"]) as tmp, \
             tc.tile_pool(name="probs", bufs=CFG["prb_bufs"]) as prb, \
             tc.tile_pool(name="ostage", bufs=CFG["ost_bufs"]) as ost, \
             tc.tile_pool(name="psA", bufs=CFG["psA_bufs"], space="PSUM") as psA, \
             tc.tile_pool(name="psS", bufs=CFG["psS_bufs"], space="PSUM") as psS, \
             tc.tile_pool(name="psAt", bufs=max(1, CFG["psAt_bufs"]), space="PSUM") as psAt:

            # ---------------- loads (all into dedicated tiles) -------------
            # order matters: the shared DMA device serializes transfers, so
            # the first qk-projection's inputs (wk + x chunk0, in ko-halves
            # so matmuls can start on the first half) go first; the RoPE
            # tables are only needed ~2 DMAs later.
            warm_in = pers.tile([128, 256], BF16, tag="warm")
            wps_pool = psA if CFG["psAt_bufs"] == 0 else psAt
            wps = wps_pool.tile([128, 256], F32,
                                tag="proj" if CFG["psAt_bufs"] == 0 else "attn",
                                name="warm_ps")

            def warmup():
                # PE warm-up: the HAM clock gate releases only after ~3.4us
                # of sustained PE activity; burn dummy matmuls on a zero tile
                # while the input DMAs are in flight so the real projections
                # run at 2.4 GHz from the start. Lowest priority: these fill
                # TensorE idle slots and keep the HAM activity window hot.
                if CFG["warm_nomemset"]:
                    # read the tile uninitialized: the product is never
                    # consumed (psum cleared by later start=True groups), and
                    # skipping the DVE memset lets PE activity - and the
                    # warm-clock ramp - start ~1.2us earlier
                    lhs, rhs = warm_in[:, 0:128], warm_in[:]
                elif CFG["warm_tiny"]:
                    with tc.high_priority():
                        nc.vector.memset(warm_in[0:1, :], 0.0)
                    lhs, rhs = warm_in[0:1, 0:128], warm_in[0:1, :]
                else:
                    nc.vector.memset(warm_in[:], 0.0)
                    lhs, rhs = warm_in[:, 0:128], warm_in[:]
                with tc.high_priority(offset=-3000000):
                    for _ in range(CFG["n_warm"]):
                        nc.tensor.matmul(wps[0:lhs.shape[1], :] if CFG["warm_tiny"] else wps[:],
                                         lhs, rhs, start=True, stop=True)
                nc.vector.memset(warm_in[0:1, 0:1], 0.0)

            if CFG["warm_first"]:
                warmup()

            xT_sb = pers.tile([128, KO, S], BF16, tag="xT")
            xTr = xT.rearrange("(ko p) s -> p ko s", p=128)
            wk_sb = pers.tile([128, KO, FPC], BF16, tag="wk")
            nc.sync.dma_start(wk_sb[:], wkp.rearrange("(ko p) m -> p ko m", p=128))

            def load_x(xc):
                for kh in range(2):
                    ks = bass.ts(kh, KO // 2)
                    nc.sync.dma_start(xT_sb[:, ks, bass.ts(xc, SQ)],
                                      xTr[:, ks, bass.ts(xc, SQ)])

            csa_sb = pers.tile([128, S], BF16, tag="csa")
            csb_sb = pers.tile([128, S], BF16, tag="csb")
            wv_sb = pers.tile([128, KO, FPC], BF16, tag="wv")
            wq_sb = pers.tile([128, KO, FPC], BF16, tag="wq")

            def load_wq():
                nc.sync.dma_start(wq_sb[:],
                                  wqp.rearrange("(ko p) m -> p ko m", p=128))

            if CFG["dma_variant"] == 4:
                # wq lands between the two x0 halves: the q projection's
                # first ko-half can start while k's second half still loads
                nc.sync.dma_start(xT_sb[:, 0:KO // 2, bass.ts(0, SQ)],
                                  xTr[:, 0:KO // 2, bass.ts(0, SQ)])
                load_wq()
                nc.sync.dma_start(xT_sb[:, KO // 2:KO, bass.ts(0, SQ)],
                                  xTr[:, KO // 2:KO, bass.ts(0, SQ)])
            else:
                load_x(0)
                load_wq()

            def load_tables():
                # the tables are 64-row periodic on-device ([cos;cos] and
                # [+s;-s]): DMA [TR, S] host-stacked rows and replicate the
                # rest with the otherwise-idle ScalarE (a scale=-1 copy
                # makes -s blocks when starting from [32, S]). ACT copy cost
                # depends on free size only, so fewer, taller copies win.
                nc.sync.dma_start(csa_sb[0:TR, :], csap[:])
                nc.sync.dma_start(csb_sb[0:TRB, :], csbp[:])
                CP = mybir.ActivationFunctionType.Copy
                # replication on the otherwise-idle ScalarE; csa/csb copies
                # INTERLEAVED (csa-r1, csb-r1, ...) so the first RoPE's
                # cos and +-sin rows both become available earliest.
                if TR == 32 and TRB == 32:
                    for r in range(1, 4):
                        sgn = -1.0 if r % 2 else 1.0
                        nc.scalar.activation(csa_sb[bass.ts(r, 32), :],
                                             csa_sb[0:32, :], CP)
                        nc.scalar.activation(csb_sb[bass.ts(r, 32), :],
                                             csb_sb[0:32, :], CP, scale=sgn)
                else:
                    if TR == 32:
                        for r in range(1, 4):
                            nc.scalar.activation(csa_sb[bass.ts(r, 32), :],
                                                 csa_sb[0:32, :], CP)
                    elif TR == 64:
                        nc.scalar.activation(csa_sb[64:128, :],
                                             csa_sb[0:64, :], CP)
                    if TRB == 32:
                        for r in range(1, 4):
                            sgn = -1.0 if r % 2 else 1.0
                            nc.scalar.activation(csb_sb[bass.ts(r, 32), :],
                                                 csb_sb[0:32, :], CP,
                                                 scale=sgn)
                    elif TRB == 64:
                        nc.scalar.activation(csb_sb[64:128, :],
                                             csb_sb[0:64, :], CP)

            def load_wv():
                nc.sync.dma_start(wv_sb[:],
                                  wvp.rearrange("(ko p) m -> p ko m", p=128))

            v = CFG["dma_variant"]
            if v == 0:
                load_tables()
                for xc in range(1, NSQ):
                    load_x(xc)
                load_wv()
            elif v == 1:
                load_x(1)
                load_tables()
                load_x(2)
                load_x(3)
                load_wv()
            elif v in (2, 4):
                load_tables()
                load_wv()
                for xc in range(1, NSQ):
                    load_x(xc)
            else:
                load_tables()
                load_wv()
                for xc in range(1, NSQ):
                    load_x(xc)
            wo_sb = pers.tile([128, FPC // 128, D], BF16, tag="wo")
            nc.sync.dma_start(wo_sb[:], wop.rearrange("(ko p) m -> p ko m", p=128))
            id_sb = pers.tile([128, 128], BF16, tag="ident")
            if CFG["pv_nat"]:
                nc.sync.dma_start(id_sb[:], idp[:])

            if not CFG["warm_first"]:
                warmup()

            # ones column for the denominator broadcast matmul
            ones_sb = pers.tile([1, DH], BF16, tag="ones")
            nc.vector.memset(ones_sb[:], 1.0)

            # persistent activations
            qT = [pers.tile([128, S], BF16, tag=f"qT{ft}", name=f"qT{ft}")
                  for ft in range(2)]
            kT = [pers.tile([128, S], BF16, tag=f"kT{ft}", name=f"kT{ft}")
                  for ft in range(2)]
            # [v | 1] as PV stationary tiles: per (sk, head) a [128, DH+1]
            v_sb = pers.tile([128, NSK, HPC, DH + 1], BF16, tag="v")
            nc.vector.memset(v_sb[:, :, :, DH:], 1.0)
            # attention output (bf16, feeds the out-projection)
            aT = [pers.tile([128, S], BF16, tag=f"aT{ft}", name=f"aT{ft}")
                  for ft in range(2)]

            # ---------------- v projection (natural layout) ---------------
            def v_proj(st):
                ps = psA.tile([128, FPC], F32, tag="proj", name="vproj_ps")
                for ko in range(KO):
                    nc.tensor.matmul(
                        ps[:],
                        xT_sb[:, ko, bass.ts(st, 128)],
                        wv_sb[:, ko, :],
                        start=(ko == 0), stop=(ko == KO - 1),
                    )
                if st < CFG["v_copy_act"]:
                    # ScalarE is idle during the start window; keeping these
                    # copies off DVE (busy with RoPE) frees psA slots sooner
                    nc.scalar.copy(
                        v_sb[:, st, :, 0:DH],
                        ps.rearrange("p (h d) -> p h d", h=HPC))
                else:
                    nc.vector.tensor_copy(
                        v_sb[:, st, :, 0:DH],
                        ps.rearrange("p (h d) -> p h d", h=HPC))

            # ---------------- q/k projection + RoPE ------------------------
            # psum rows per head offset: [t0 (32) ; t1 (32)]. One PSUM->SBUF
            # bf16 copy, then 6 SBUF ops at the DVE 2x rate:
            #   mc       = qs * cos_rep                       (128 rows)
            #   msx[ 0:32 ] = qs[32:64 ] * csb[32:64 ]  (= -t1*s: csb row
            #   msx[32:64 ] = qs[ 0:32 ] * csb[ 0:32 ]   blocks alternate
            #   msx[64:96 ] = qs[96:128] * csb[96:128]   +s,-s,+s,-s so all
            #   msx[96:128] = qs[64:96 ] * csb[64:96 ]   combines are ADDs)
            #   dst      = mc + msx                           (128 rows)
            # (partition patterns at base 32/96 are limited to 32 partitions
            # on this walrus, hence the 32-aligned strips; both SBUF inputs
            # of a TensorTensor must share their base partition, the output
            # may differ)
            def qk_proj(w_sb, dst, ft, sq, split_heads=False):
                sl = bass.ts(sq, SQ)
                ps = psA.tile([128, SQ], F32, tag="proj", name="qkproj_ps")
                for ko in range(KO):
                    nc.tensor.matmul(
                        ps[:],
                        w_sb[:, ko, bass.ts(ft, 128)],
                        xT_sb[:, ko, bass.ts(sq, SQ)],
                        start=(ko == 0), stop=(ko == KO - 1),
                    )
                qs = tmp.tile([128, SQ], BF16, tag="ropeQS")
                nc.vector.tensor_copy(qs[:], ps[:])
                mc = tmp.tile([128, SQ], BF16, tag="ropeMC")
                msx = tmp.tile([128, SQ], BF16, tag="ropeMSX")
                if split_heads:
                    # per-head chains so the first head's scores can issue
                    # before the second head's RoPE finishes (start latency)
                    nc.vector.tensor_mul(mc[0:64, :], qs[0:64, :],
                                         csa_sb[0:64, sl])
                    nc.vector.tensor_mul(msx[0:32, :], qs[32:64, :],
                                         csb_sb[32:64, sl])
                    nc.vector.tensor_mul(msx[32:64, :], qs[0:32, :],
                                         csb_sb[0:32, sl])
                    nc.vector.tensor_add(dst[0:64, sl], mc[0:64, :],
                                         msx[0:64, :])
                    nc.vector.tensor_mul(mc[64:128, :], qs[64:128, :],
                                         csa_sb[64:128, sl])
                    nc.vector.tensor_mul(msx[64:96, :], qs[96:128, :],
                                         csb_sb[96:128, sl])
                    nc.vector.tensor_mul(msx[96:128, :], qs[64:96, :],
                                         csb_sb[64:96, sl])
                    nc.vector.tensor_add(dst[64:128, sl], mc[64:128, :],
                                         msx[64:128, :])
                    return
                nc.vector.tensor_mul(mc[:], qs[:], csa_sb[:, sl])
                nc.vector.tensor_mul(msx[0:32, :], qs[32:64, :],
                                     csb_sb[32:64, sl])
                nc.vector.tensor_mul(msx[32:64, :], qs[0:32, :],
                                     csb_sb[0:32, sl])
                nc.vector.tensor_mul(msx[64:96, :], qs[96:128, :],
                                     csb_sb[96:128, sl])
                nc.vector.tensor_mul(msx[96:128, :], qs[64:96, :],
                                     csb_sb[64:96, sl])
                nc.vector.tensor_add(dst[:, sl], mc[:], msx[:])

            # ---------------- attention block ------------------------------
            def attn_block(sq, h, n_dve_exp=0, bs_act=True, col_split=False):
                sl = bass.ts(sq, SQ)
                ft, off = h // 2, (h % 2) * 64
                if CFG["pv_nat"]:
                    at = None   # natural-PV path allocates its own psum
                elif CFG["merge_at_bc"]:
                    # one 128-partition tile per block: PV accumulates into
                    # rows 0..64 and the ones-broadcast matmul reuses rows
                    # 64..127 (the reciprocal reads the denominator row
                    # before the broadcast overwrites it). Keeps bc from
                    # occupying a second psAt slot, so block n+1's PV can
                    # start while block n's normalize still runs.
                    at = psAt.tile([128, SQ], F32, tag="attn")
                else:
                    at = psAt.tile([DH + 1, SQ], F32, tag="attn")
                # spread the DVE-exp'd tiles across the block
                dve_tiles = {NSK // 2 - 1 - 2 * j for j in range(n_dve_exp)}
                def emit_sc_exp_g(sks, dve):
                    # one score tile + ONE exp instruction for a GROUP of
                    # key tiles (bigger groups amortize the per-exp access
                    # overhead and slot-recycle pitch on ScalarE)
                    g = len(sks)
                    GW = CFG["sk_group"]
                    sc = psS.tile([128, GW, SQ], F32, tag="sc")
                    pb = prb.tile([128, GW, SQ], BF16, tag="pb")
                    for i, sk in enumerate(sks):
                        nc.tensor.matmul(
                            sc[:, i, :],
                            kT[ft][off:off + 64, bass.ts(sk, SK)],
                            qT[ft][off:off + 64, sl],
                            start=True, stop=True,
                        )
                    if dve:
                        with nc.allow_low_precision(reason="fast exp"):
                            nc.vector.tensor_scalar(
                                pb[:, 0:g, :].bitcast(I16), sc[:, 0:g, :],
                                EXPA, EXPB,
                                mybir.AluOpType.mult, mybir.AluOpType.add)
                    else:
                        nc.scalar.activation(
                            pb[:, 0:g, :], sc[:, 0:g, :],
                            mybir.ActivationFunctionType.Exp, scale=EXP_SCALE)
                    return pb

                def emit_sc_exp(sk2):
                    return emit_sc_exp_g([2 * sk2, 2 * sk2 + 1],
                                         sk2 in dve_tiles)

                def emit_pv(sk2, pb):
                    for i in range(2):
                        sk = 2 * sk2 + i
                        nc.tensor.matmul(
                            at[0:DH + 1, :], v_sb[:, sk, h, :], pb[:, i, :],
                            start=(sk == 0), stop=(sk == NSK - 1),
                        )

                def emit_pv_nat(sk2, pb, atn):
                    # natural-layout PV: probs is the STATIONARY operand so
                    # the output is [128 queries, DH+1] -- free size 65
                    # instead of 512, 4x cheaper on TensorE per element.
                    # PSUM start=True zeroes the WHOLE 2KB bank
                    # (ZERO_REGION_SIZE), so only the very first matmul may
                    # carry it: the other query-subtiles' first writes
                    # accumulate onto the already-zeroed bank.
                    for i in range(2):
                        sk = 2 * sk2 + i
                        for qs4 in range(4):
                            nc.tensor.matmul(
                                atn[:, qs4, :],
                                pb[:, i, bass.ts(qs4, 128)],
                                v_sb[:, sk, h, :],
                                start=(sk == 0 and qs4 == 0),
                                stop=(sk == NSK - 1),
                                skip_group_check=True,
                            )

                if CFG["pv_nat"]:
                    atn_pool = psA if CFG["psAt_bufs"] == 0 else psAt
                    atn = atn_pool.tile(
                        [128, 4, DH + 1], F32,
                        tag="proj" if CFG["psAt_bufs"] == 0 else "attn",
                        name="at_nat")
                    GW = CFG["sk_group"]
                    groups = [list(range(s, min(s + GW, NSK)))
                              for s in range(0, NSK, GW)]
                    for gi, sks in enumerate(groups):
                        pbs = emit_sc_exp_g(sks, False)
                        for i, sk in enumerate(sks):
                            for qs4 in range(4):
                                nc.tensor.matmul(
                                    atn[:, qs4, :],
                                    pbs[:, i, bass.ts(qs4, 128)],
                                    v_sb[:, sk, h, :],
                                    start=(sk == 0 and qs4 == 0),
                                    stop=(sk == NSK - 1),
                                    skip_group_check=True,
                                )
                    for qs4 in range(4):
                        # per-partition normalize (queries on partitions):
                        # no broadcast needed at all
                        rcn = tmp.tile([128, 1], F32, tag="recip", name="rcn")
                        with nc.allow_low_precision(
                                reason="softmax denominator"):
                            nc.vector.reciprocal(
                                rcn[:], atn[:, qs4, DH:DH + 1])
                        ann = tmp.tile([128, DH], BF16, tag="anat",
                                       name="ann")
                        nc.vector.tensor_scalar(
                            ann[:], atn[:, qs4, 0:DH], rcn[:], None,
                            mybir.AluOpType.mult)
                        # transpose back to [features, queries] for the
                        # out-projection (PE transpose mode, bf16)
                        use_psA = CFG["tp_psA"] or CFG["psAt_bufs"] == 0
                        tp = (psA if use_psA else psAt).tile(
                            [DH, 128], BF16,
                            tag="proj" if use_psA else "attn",
                            name="tp_ps")
                        nc.tensor.transpose(tp[:], ann[:], id_sb[:])
                        csl = bass.ts(4 * sq + qs4, 128)
                        if bs_act:
                            nc.scalar.copy(aT[ft][off:off + 64, csl], tp[:])
                        else:
                            nc.vector.tensor_copy(aT[ft][off:off + 64, csl],
                                                  tp[:])
                    return
                if CFG["attn_pipe"]:
                    # software-pipelined emission: next tile's scores sit
                    # ahead of this tile's PV in the tie-break order
                    pbs = emit_sc_exp(0)
                    for sk2 in range(1, NSK // 2):
                        pb_next = emit_sc_exp(sk2)
                        emit_pv(sk2 - 1, pbs)
                        pbs = pb_next
                    emit_pv(NSK // 2 - 1, pbs)
                else:
                    for sk2 in range(NSK // 2):
                        pbs = emit_sc_exp(sk2)
                        emit_pv(sk2, pbs)
                ctx = tc.high_priority(offset=CFG["norm_prio"]) \
                    if CFG["norm_prio"] else None
                if ctx is not None:
                    ctx.__enter__()
                # col_split: run the normalize per column half so the first
                # half of the (column-split) out-projection can start while
                # the second half still normalizes -- shortens the epilogue
                # of the final attention block.
                SH = SQ // 2
                halves = ((0, SH), (SH, SH)) if col_split else ((0, SQ),)
                for c0, cw in halves:
                    cs = slice(c0, c0 + cw)
                    sls = bass.ts(2 * sq + c0 // SH, SH) if col_split else sl
                    rc = tmp.tile([1, cw], BF16, tag="recip", name="rc")
                    with nc.allow_low_precision(reason="softmax denominator"):
                        nc.vector.reciprocal(rc[:], at[DH:DH + 1, cs])
                    if CFG["merge_at_bc"]:
                        bc = at[DH:2 * DH, cs]
                        nc.tensor.matmul(bc, ones_sb[:], rc[:],
                                         start=True, stop=True,
                                         skip_group_check=True)
                    else:
                        bct = psAt.tile([DH, cw], F32, tag="attn",
                                        name="bcast_ps")
                        bc = bct[:]
                        nc.tensor.matmul(bc, ones_sb[:], rc[:],
                                         start=True, stop=True)
                    bs = tmp.tile([DH, cw], F32, tag="bcsb", name="bs")
                    if bs_act:
                        nc.scalar.copy(bs[:], bc)
                    else:
                        nc.vector.tensor_copy(bs[:], bc)
                    nc.vector.tensor_mul(aT[ft][off:off + 64, sls],
                                         at[0:DH, cs], bs[:])
                if ctx is not None:
                    ctx.__exit__(None, None, None)

            # ---------------- out-projection for one query block -----------
            outpR = outp.rearrange("(fo p) s -> p fo s", p=128)

            def out_proj_finish(sq, fo, po, stg=None):
                sl = bass.ts(sq, SQ)
                on_act = fo < CFG["stage_act"]
                if sq == NSQ - 1 and fo % 2 == 1 and \
                        fo < 2 * CFG["stage_act_last"]:
                    on_act = True
                if stg is None:
                    stg1 = ost.tile([128, SQ], BF16, tag="oT", name="stg1")
                    dst = stg1[:]
                else:
                    stg1 = None
                    dst = stg
                if on_act:
                    nc.scalar.copy(dst, po[:])
                else:
                    nc.vector.tensor_copy(dst, po[:])
                if stg1 is not None:
                    nc.sync.dma_start(outp[bass.ts(fo, 128), sl], dst)

            def out_proj_ko(sq, fo, po, ko, start, stop):
                nc.tensor.matmul(
                    po[:],
                    wo_sb[:, ko, bass.ts(fo, 128)],
                    aT[ko][:, bass.ts(sq, SQ)],
                    start=start, stop=stop,
                )

            def out_proj(sq, skip_fo=()):
                last = sq == NSQ - 1
                pair = CFG["dma_pairs"]
                csplit = last and CFG["col_split_last"]
                sl = bass.ts(sq, SQ)
                SH = SQ // 2
                stg2 = None
                for fo in range(8):
                    if fo in skip_fo:
                        continue
                    # on the last block the scores stream is done, so its
                    # PSUM pool is free: borrow it for 2 extra po slots
                    if last and fo % 2 == 1:
                        po = psS.tile([128, SQ], F32, tag="sc", name="oproj_ps2")
                    else:
                        po = psA.tile([128, SQ], F32, tag="proj", name="oproj_ps")
                    if csplit:
                        # column-split: the first half contracts aT columns
                        # that finish normalizing earlier
                        for ch in range(2):
                            ccs = slice(ch * SH, (ch + 1) * SH)
                            for ko in range(2):
                                nc.tensor.matmul(
                                    po[:, ccs],
                                    wo_sb[:, ko, bass.ts(fo, 128)],
                                    aT[ko][:, bass.ts(2 * sq + ch, SH)],
                                    start=(ko == 0), stop=(ko == 1),
                                )
                    else:
                        out_proj_ko(sq, fo, po, 0, True, False)
                        out_proj_ko(sq, fo, po, 1, False, True)
                    if not pair:
                        out_proj_finish(sq, fo, po)
                        continue
                    # paired staging: two fo blocks share one [128,2,SQ]
                    # tile and ONE output DMA (halves the serial HWDGE
                    # descriptor-generation slots that pace the tail)
                    if fo % 2 == 0:
                        stg2 = ost.tile([128, 2, SQ], BF16, tag="oT")
                        dsts = stg2[:, 0, :]
                    else:
                        dsts = stg2[:, 1, :]
                    if csplit:
                        # stage per column half (alternating engines) so
                        # the first half's copy runs during the second
                        # half's matmuls
                        for ch in range(2):
                            ccs = slice(ch * SH, (ch + 1) * SH)
                            if (fo + ch) % 2 == 0:
                                nc.vector.tensor_copy(dsts[:, ccs],
                                                      po[:, ccs])
                            else:
                                nc.scalar.copy(dsts[:, ccs], po[:, ccs])
                    else:
                        out_proj_finish(sq, fo, po, stg=dsts)
                    if fo % 2 == 1:
                        nc.sync.dma_start(outpR[:, fo - 1:fo + 1, sl],
                                          stg2[:])

            # ---------------- emission order (overlap) ---------------------
            if CFG["emit_variant"] == 0:
                # two head-passes: heads 0,1 for all sq, then 2,3 + out-proj
                qk_proj(wk_sb, kT[0], 0, 0, split_heads=CFG["rope_split0"])
                qk_proj(wq_sb, qT[0], 0, 0, split_heads=CFG["rope_split0"])
                for sq in range(1, NSQ):
                    qk_proj(wk_sb, kT[0], 0, sq)
                with tc.high_priority(offset=-400000):
                    for sq in range(1, NSQ):
                        qk_proj(wq_sb, qT[0], 0, sq)
                with tc.high_priority(offset=CFG["v_prio"]):
                    for st in range(8):
                        v_proj(st)
                with tc.high_priority(offset=-1000000):
                    for st in range(8, NSK):
                        v_proj(st)
                for sq in range(NSQ):
                    attn_block(sq, 0, CFG["dve_exp_p1"], CFG["bs_act_p1"])
                    attn_block(sq, 1, CFG["dve_exp_p1"], CFG["bs_act_p1"])
                with tc.high_priority(offset=-500000):
                    for sq in range(NSQ):
                        qk_proj(wk_sb, kT[1], 1, sq)
                    for sq in range(NSQ):
                        qk_proj(wq_sb, qT[1], 1, sq)
                for sq in range(NSQ):
                    attn_block(sq, 2, CFG["dve_exp_p2"], CFG["bs_act_p2"])
                    attn_block(sq, 3, CFG["dve_exp_p2"], CFG["bs_act_p2"])
                    with tc.high_priority(offset=CFG["out_prio"]):
                        out_proj(sq)
            else:
                # per-sq: all 4 heads of each query block back-to-back, with
                # the next block's projections + v + out-proj as PE fill --
                # balances the ACT exp stream across the whole kernel span.
                qk_proj(wk_sb, kT[0], 0, 0, split_heads=CFG["rope_split0"])
                qk_proj(wq_sb, qT[0], 0, 0, split_heads=CFG["rope_split0"])
                with tc.high_priority(offset=CFG["qk_ahead_prio"]):
                    qk_proj(wk_sb, kT[1], 1, 0)
                    qk_proj(wq_sb, qT[1], 1, 0)
                with tc.high_priority(offset=CFG["v_prio"]):
                    for st in range(8):
                        v_proj(st)
                with tc.high_priority(offset=-1000000):
                    for st in range(8, NSK):
                        v_proj(st)
                ho = CFG["head_order"]
                for sq in range(NSQ):
                    last = sq == NSQ - 1
                    nsplit = CFG["out_split_last"] if last else 0
                    de1 = CFG["dve_exp_last01"] if last else CFG["dve_exp_p1"]
                    attn_block(sq, ho[0], de1, CFG["bs_act_p1"])
                    attn_block(sq, ho[1], de1, CFG["bs_act_p1"])
                    if sq + 1 < NSQ:
                        with tc.high_priority(offset=CFG["qk_ahead_prio"]):
                            qk_proj(wk_sb, kT[0], 0, sq + 1)
                            qk_proj(wq_sb, qT[0], 0, sq + 1)
                    # last sq: psA is otherwise idle now, so pre-accumulate
                    # the ko0 half (reads aT[0] = heads 0,1, already final)
                    # of the first fo groups; only ko1+stage+DMA remain
                    # after the last head's normalize.
                    pre = []
                    for fo in range(nsplit):
                        po = psA.tile([128, SQ], F32, tag="proj",
                                      name="oproj_ps")
                        out_proj_ko(sq, fo, po, 0, True, False)
                        pre.append((fo, po))
                    de2 = CFG["dve_exp_last"] if last else CFG["dve_exp_p2"]
                    ncs = CFG["col_split_last"] if last else 0
                    attn_block(sq, ho[2], de2, CFG["bs_act_p2"],
                               col_split=(ncs >= 2))
                    attn_block(sq, ho[3], de2, CFG["bs_act_p2"],
                               col_split=(ncs >= 1))
                    if sq + 1 < NSQ:
                        with tc.high_priority(offset=CFG["qk_ahead_prio"]):
                            qk_proj(wk_sb, kT[1], 1, sq + 1)
                            qk_proj(wq_sb, qT[1], 1, sq + 1)
                    with tc.high_priority(offset=CFG["out_prio"]):
                        for fo, po in pre:
                            out_proj_ko(sq, fo, po, 1, False, True)
                            out_proj_finish(sq, fo, po)
                        out_proj(sq, skip_fo=tuple(f for f, _ in pre))

    _split_multi_waits(nc, keep=CFG["wsplit_keep"])
    return nc


_NC_CACHE = None


def _get_nc():
    global _NC_CACHE
    if _NC_CACHE is None:
        _NC_CACHE = _build()
    return _NC_CACHE


# rotation-pair permutation: within each head, [0,2,...,62, 1,3,...,63]
_PAIR_PERM = np.concatenate([np.arange(0, DH, 2), np.arange(1, DH, 2)])


def kernel(x, freqs_cos, freqs_sin, wq, wk, wv, wo):
    x = np.asarray(x, dtype=np.float32)
    cosT = np.asarray(freqs_cos, np.float32).T    # [32, S]
    sinT = np.asarray(freqs_sin, np.float32).T
    # host-stacked base tables (the kernel replicates the rest on-device;
    # the on-device pattern is [cos]x4 and [+s,-s,+s,-s] per 32-row block)
    TR = CFG["table_rows"]
    TRB = max(TR, CFG["table_rows_b"])
    csa = np.ascontiguousarray(
        np.concatenate([cosT] * (TR // 32), 0)).astype(ml_dtypes.bfloat16)
    sgn = [sinT if r % 2 == 0 else -sinT for r in range(TRB // 32)]
    csb = np.ascontiguousarray(np.concatenate(sgn, 0)).astype(
        ml_dtypes.bfloat16)
    wq = np.asarray(wq, np.float32)
    wk = np.asarray(wk, np.float32)
    wv = np.asarray(wv, np.float32)
    wo = np.asarray(wo, np.float32)

    bf = ml_dtypes.bfloat16
    in_maps = []
    for c in range(N_CORES):
        b, hg = divmod(c, N_CORES // B)
        heads = [hg * HPC + i for i in range(HPC)]
        qk_cols = np.concatenate([h * DH + _PAIR_PERM for h in heads])
        v_cols = np.concatenate([h * DH + np.arange(DH) for h in heads])
        in_maps.append({
            "ident": np.eye(128, dtype=np.float32).astype(bf),
            "xT": np.ascontiguousarray(x[b].T).astype(bf),
            "wq": np.ascontiguousarray(wq[:, qk_cols]).astype(bf),
            "wk": np.ascontiguousarray(wk[:, qk_cols]).astype(bf),
            "wv": np.ascontiguousarray(wv[:, v_cols]).astype(bf),
            "wo": np.ascontiguousarray(wo[v_cols, :]).astype(bf),
            "csa": csa,
            "csb": csb,
        })

    nc = _get_nc()
    res = run_bass_kernel_spmd(nc, in_maps, core_ids=list(range(N_CORES)))

    out = np.zeros((B, S, D), dtype=np.float32)
    for c in range(N_CORES):
        b = c // (N_CORES // B)
        out[b] += res.results[c]["outT"].astype(np.float32).T
    return out

